# revision 1
# baseline (speedup 1.0000x reference)
"""Trainium2 Bass kernel for nn_Graph_CNN_Feat_Mesh (Chebyshev GNN decoder).

Strategy (per-core, data-parallel over batch B=256 -> 32/core):
  - All spmms are dense matmuls on the tensor engine (PE) in bf16:
      y = A + L @ (B + L @ (2C)),  A/B/C = feature-space linears of the input.
    L is densified on host; for up4-preceded layers the replication is folded
    into LU = L @ U (contracting the small pre-upsample vertex space).
  - B and A linear terms accumulate directly into the spmm PSUM.
  - Activations live in packed F-layout [(j,Fin) partitions, (b//G)*Vsp + v]
    between layers; the per-layer linear emits V-layout directly; one PE
    transpose per layer returns to F-layout.
  - BatchNorm (training mode, global batch stats) is exact: per-core partial
    sums are AllReduce'd across the 8 cores in-kernel; scale/shift+relu fused
    into one ScalarE activation per layer.
  - FC head (2048->512->5120) runs in fp32; graph layers use bf16 operands
    with fp32 PSUM accumulation.
"""

import numpy as np

B = 256
NCORES = 8
BL = B // NCORES  # 32
EPS = 1e-5

_CACHE = {}


def _split_W(W):
    W = np.asarray(W, np.float32)
    return W[:, 0::3], W[:, 1::3], W[:, 2::3]


def _dense_L(rows, cols, vals, V):
    L = np.zeros((V, V), np.float32)
    np.add.at(L, (np.asarray(rows), np.asarray(cols)), np.asarray(vals, np.float32))
    return L


def _pad_rows(a, m):
    if a.shape[0] % m == 0:
        return a
    p = m - a.shape[0] % m
    return np.concatenate([a, np.zeros((p,) + a.shape[1:], a.dtype)], 0)


class _LCfg:
    def __init__(self, name, Vsp, V, Fin, Fout, up4, bn):
        self.name = name
        self.Vsp = Vsp      # source vertex space of C-linear (pre-up4)
        self.V = V          # output vertex count
        self.Fin = Fin
        self.Fout = Fout
        self.G = 128 // Fin          # batches packed on partitions at input
        self.nG = BL // self.G
        self.GF = self.G * Fout      # N of one B/C/A-linear matmul
        self.Gp = 128 // Fout if Fout in (32, 64) else None
        self.nGp = BL // self.Gp if self.Gp else None
        self.up4 = up4
        self.bn = bn
        self.nVt = (V + 127) // 128
        self.nVsp = (Vsp + 127) // 128
        self.BF = BL * Fout          # free width of V-layout per vtile

    def vts(self, t):
        return min(128, self.V - t * 128)

    def sps(self, s):
        return min(128, self.Vsp - s * 128)


CFGS = [
    _LCfg("c0", 80, 320, 64, 64, True, True),
    _LCfg("c1", 320, 320, 64, 32, False, True),
    _LCfg("c2", 320, 1280, 32, 32, True, True),
    _LCfg("c3", 1280, 1280, 32, 3, False, False),
]


def _wbd(W, G, Fin, Fout, which):
    """Block-diagonal rhs weight [128, G*Fout] for the fused linear.
    which: 'A' -> W0 - W2, 'B' -> W1, 'C' -> 2*W2.  col = j*Fout + c."""
    W0, W1, W2 = _split_W(W)
    M = {"A": W0 - W2, "B": W1, "C": 2.0 * W2}[which]  # [Fout, Fin]
    out = np.zeros((128, G * Fout), np.float32)
    for j in range(G):
        out[j * Fin:(j + 1) * Fin, j * Fout:(j + 1) * Fout] = M.T
    return out


def _build_host(inputs):
    import ml_dtypes
    bf = ml_dtypes.bfloat16
    f32 = np.float32
    d = {}
    d["xT"] = np.ascontiguousarray(np.asarray(inputs["x"], f32).T)  # [2048, 256]
    d["fc1wT"] = np.ascontiguousarray(np.asarray(inputs["fc1_w"], f32).T)
    d["fc1b"] = np.ascontiguousarray(
        np.asarray(inputs["fc1_b"], f32).reshape(4, 128).T)  # [128,4]
    d["fc2wT"] = np.ascontiguousarray(np.asarray(inputs["fc2_w"], f32).T)

    L1 = _dense_L(inputs["L1_rows"], inputs["L1_cols"], inputs["L1_vals"], 320)
    L2 = _dense_L(inputs["L2_rows"], inputs["L2_cols"], inputs["L2_vals"], 1280)
    U1 = np.repeat(np.eye(80, dtype=f32), 4, axis=0)    # [320, 80]
    U2 = np.repeat(np.eye(320, dtype=f32), 4, axis=0)   # [1280, 320]
    d["LU0"] = _pad_rows(np.ascontiguousarray((L1 @ U1).T), 128).astype(bf)  # [128,320]
    d["LT1"] = _pad_rows(np.ascontiguousarray(L1.T), 128).astype(bf)         # [384,320]
    d["LU2"] = _pad_rows(np.ascontiguousarray((L2 @ U2).T), 128).astype(bf)  # [384,1280]
    d["LT2"] = np.ascontiguousarray(L2.T).astype(bf)                         # [1280,1280]

    Wn = {"c0": "cl0_w", "c1": "cl1_w", "c2": "cl2_w", "c3": "cl3_w"}
    for cfg in CFGS:
        W = np.asarray(inputs[Wn[cfg.name]], f32)
        for which in "ABC":
            d[f"W{which}_{cfg.name}"] = _wbd(
                W, cfg.G, cfg.Fin, cfg.Fout, which).astype(bf)
    d["b3"] = np.asarray(inputs["cl3_b"], f32).copy()

    for i, (g, b) in enumerate([("bn0_g", "bn0_b"), ("bn1_g", "bn1_b"),
                                ("bn2_g", "bn2_b")]):
        gb = np.concatenate([np.asarray(inputs[g], f32),
                             np.asarray(inputs[b], f32)])
        d[f"gb{i}"] = np.ascontiguousarray(gb[None, :])  # [1, 2F]

    for F, nm in [(64, "sel64"), (32, "sel32")]:
        Gp = 128 // F
        sel = np.zeros((128, F), f32)
        for j in range(Gp):
            sel[j * F:(j + 1) * F] += np.eye(F, dtype=f32)
        d[nm] = sel
    return d


def _build_nc(b3_imm):
    import sys
    for p in ("/opt/trn_rl_repo", "/opt/trn_rl_repo/concourse"):
        if p not in sys.path:
            sys.path.insert(0, p)
    import concourse.bass as bass  # noqa
    import concourse.mybir as mybir
    import concourse.tile as tile
    from concourse import bacc
    from concourse.masks import make_identity

    f32 = mybir.dt.float32
    bf16 = mybir.dt.bfloat16
    AF = mybir.ActivationFunctionType
    ALU = mybir.AluOpType

    nc = bacc.Bacc(None, target_bir_lowering=False)

    xT = nc.dram_tensor("xT", [2048, BL], f32, kind="ExternalInput")
    fc1wT = nc.dram_tensor("fc1wT", [2048, 512], f32, kind="ExternalInput")
    fc1b = nc.dram_tensor("fc1b", [128, 4], f32, kind="ExternalInput")
    fc2wT = nc.dram_tensor("fc2wT", [512, 5120], f32, kind="ExternalInput")
    LU0 = nc.dram_tensor("LU0", [128, 320], bf16, kind="ExternalInput")
    LT1 = nc.dram_tensor("LT1", [384, 320], bf16, kind="ExternalInput")
    LU2 = nc.dram_tensor("LU2", [384, 1280], bf16, kind="ExternalInput")
    LT2 = nc.dram_tensor("LT2", [1280, 1280], bf16, kind="ExternalInput")
    Wt = {}
    for cfg in CFGS:
        for w in "ABC":
            Wt[f"{w}{cfg.name}"] = nc.dram_tensor(
                f"W{w}_{cfg.name}", [128, cfg.GF], bf16, kind="ExternalInput")
    gbs = [nc.dram_tensor(f"gb{i}", [1, 2 * F], f32, kind="ExternalInput")
           for i, F in enumerate([64, 32, 32])]
    sel64 = nc.dram_tensor("sel64", [128, 64], f32, kind="ExternalInput")
    sel32 = nc.dram_tensor("sel32", [128, 32], f32, kind="ExternalInput")
    ydram = nc.dram_tensor("y", [BL, 1280 * 3], f32, kind="ExternalOutput")

    with tile.TileContext(nc) as tc:
        with (
            tc.tile_pool(name="const", bufs=1) as constp,
            tc.tile_pool(name="wpool", bufs=1) as wpool,
            tc.tile_pool(name="poolA", bufs=2) as poolA,
            tc.tile_pool(name="poolB", bufs=2) as poolB,
            tc.tile_pool(name="poolC", bufs=1) as poolC,
            tc.tile_pool(name="misc", bufs=1) as miscp,
            tc.tile_pool(name="outp", bufs=3) as outp,
            tc.tile_pool(name="pslin", bufs=2, space="PSUM") as pslin,
            tc.tile_pool(name="psbig", bufs=2, space="PSUM") as psbig,
            tc.tile_pool(name="pstr", bufs=2, space="PSUM") as pstr,
            tc.tile_pool(name="dram", bufs=1, space="DRAM") as dramp,
        ):
            # ---- constants ----
            ident_b = constp.tile([128, 128], bf16, tag="identb")
            make_identity(nc, ident_b[:])
            ident_f = constp.tile([128, 128], f32, tag="identf")
            make_identity(nc, ident_f[:])
            sel_sb = {64: constp.tile([128, 64], f32, tag="sel64", name="sel64sb"),
                      32: constp.tile([128, 32], f32, tag="sel32", name="sel32sb")}
            nc.sync.dma_start(sel_sb[64][:], sel64[:])
            nc.sync.dma_start(sel_sb[32][:], sel32[:])
            gb_sb = []
            for i, F in enumerate([64, 32, 32]):
                t = constp.tile([1, 2 * F], f32, tag=f"gb{i}")
                nc.sync.dma_start(t[:], gbs[i][:])
                gb_sb.append(t)
            fc1b_sb = constp.tile([128, 4], f32, tag="fc1b")
            nc.sync.dma_start(fc1b_sb[:], fc1b[:])
            eps_t = constp.tile([1, 1], f32, tag="eps")
            nc.gpsimd.memset(eps_t[:], EPS)

            # L matrices + cheby weights
            LUT, LT = {}, {}
            t = wpool.tile([128, 320], bf16, tag="LU0")
            nc.sync.dma_start(t[:], LU0[0:128, :])
            LUT["c0"] = t
            t = wpool.tile([128, 3 * 320], bf16, tag="LT1")
            for s in range(3):
                nc.sync.dma_start(t[:, s * 320:(s + 1) * 320],
                                  LT1[s * 128:(s + 1) * 128, :])
            LT["c0"] = LT["c1"] = LUT["c1"] = t
            t = wpool.tile([128, 3 * 1280], bf16, tag="LU2")
            for s in range(3):
                nc.sync.dma_start(t[:, s * 1280:(s + 1) * 1280],
                                  LU2[s * 128:(s + 1) * 128, :])
            LUT["c2"] = t
            t = wpool.tile([128, 10 * 1280], bf16, tag="LT2")
            for s in range(10):
                nc.sync.dma_start(t[:, s * 1280:(s + 1) * 1280],
                                  LT2[s * 128:(s + 1) * 128, :])
            LT["c2"] = LT["c3"] = LUT["c3"] = t
            W_sb = {}
            for cfg in CFGS:
                for w in "ABC":
                    ti = wpool.tile([128, cfg.GF], bf16, tag=f"W{w}{cfg.name}")
                    nc.sync.dma_start(ti[:], Wt[f"{w}{cfg.name}"][:])
                    W_sb[f"{w}{cfg.name}"] = ti

            # ================= FC head (fp32) =================
            xT_sb = miscp.tile([128, 16 * BL], f32, tag="xT")
            for kt in range(16):
                nc.sync.dma_start(xT_sb[:, kt * BL:(kt + 1) * BL],
                                  xT[kt * 128:(kt + 1) * 128, :])
            fc1w_sb = poolA.tile([128, 16 * 512], f32, tag="A")
            for kt in range(16):
                nc.sync.dma_start(fc1w_sb[:, kt * 512:(kt + 1) * 512],
                                  fc1wT[kt * 128:(kt + 1) * 128, :])

            h1T = miscp.tile([128, 4 * BL], f32, tag="h1T")
            ps1 = pslin.tile([128, 4 * BL], f32, tag="lin")
            for mt in range(4):
                for kt in range(16):
                    nc.tensor.matmul(
                        ps1[:, mt * BL:(mt + 1) * BL],
                        fc1w_sb[:, kt * 512 + mt * 128: kt * 512 + (mt + 1) * 128],
                        xT_sb[:, kt * BL:(kt + 1) * BL],
                        start=(kt == 0), stop=(kt == 15))
                nc.scalar.activation(
                    h1T[:, mt * BL:(mt + 1) * BL], ps1[:, mt * BL:(mt + 1) * BL],
                    AF.Relu, bias=fc1b_sb[:, mt:mt + 1])

            # fc2 streamed in 4 column-chunks of 1280 (10 m-tiles each).
            # psum partition = (v0%2)*64+f, col = mi*BL+b ; channels c = v0*64+f.
            # dest: XF0[(b%2)*64+f, (b//2)*80 + v0],  v0 = 2*(mc*10+mi)+p0
            XF0 = poolC.tile([128, 16 * 80], bf16, tag="XF0")
            for mc in range(4):
                wch = poolB.tile([128, 4 * 1280], f32, tag="B")
                for kt in range(4):
                    nc.sync.dma_start(
                        wch[:, kt * 1280:(kt + 1) * 1280],
                        fc2wT[kt * 128:(kt + 1) * 128,
                              mc * 1280:(mc + 1) * 1280])
                ps2 = psbig.tile([128, 10 * BL], f32, tag="big")
                for mi in range(10):
                    for kt in range(4):
                        nc.tensor.matmul(
                            ps2[:, mi * BL:(mi + 1) * BL],
                            wch[:, kt * 1280 + mi * 128: kt * 1280 + (mi + 1) * 128],
                            h1T[:, kt * BL:(kt + 1) * BL],
                            start=(kt == 0), stop=(kt == 3))
                src4 = ps2[:].rearrange("p (i g j) -> p i g j", g=16, j=2)
                dst4 = XF0[:].rearrange("p (g u q) -> p g u q", u=40, q=2)
                for p0 in range(2):
                    for j in range(2):
                        nc.scalar.activation(
                            dst4[j * 64:(j + 1) * 64, :,
                                 mc * 10:(mc + 1) * 10, p0]
                            .rearrange("p g i -> p i g"),
                            src4[p0 * 64:(p0 + 1) * 64, :, :, j],
                            AF.Copy)

            # ================= cheby layers =================
            XF_cur = XF0
            ar_idx = 0

            for li, cfg in enumerate(CFGS):
                V, Vsp, F = cfg.V, cfg.Vsp, cfg.Fout
                BF = cfg.BF
                last = cfg.name == "c3"

                # --- replicate input for B/A linears if up4 ---
                if cfg.up4:
                    XFrep = poolA.tile([128, cfg.nG * V], bf16, tag="A")
                    s_r = XF_cur[:].rearrange("p (g w) -> p g w", w=Vsp)
                    d_r = XFrep[:].rearrange("p (g w r) -> p g w r", w=Vsp, r=4)
                    for r in range(4):
                        nc.vector.tensor_copy(d_r[:, :, :, r], s_r)
                else:
                    XFrep = XF_cur

                # --- C linear (in Vsp space) ---
                XC = poolC.tile([128, cfg.nVsp * BL * F], bf16, tag="XC")
                gpack = max(1, 512 // cfg.GF)
                for s in range(cfg.nVsp):
                    ssz = cfg.sps(s)
                    for g0 in range(0, cfg.nG, gpack):
                        gn = min(gpack, cfg.nG - g0)
                        pc = pslin.tile([128, 512], f32, tag="lin")
                        for gi in range(gn):
                            g = g0 + gi
                            nc.tensor.matmul(
                                pc[:ssz, gi * cfg.GF:(gi + 1) * cfg.GF],
                                XF_cur[:, g * Vsp + s * 128:
                                       g * Vsp + s * 128 + ssz],
                                W_sb[f"C{cfg.name}"][:],
                                start=True, stop=True)
                        nc.scalar.activation(
                            XC[:ssz, s * BL * F + g0 * cfg.GF:
                               s * BL * F + (g0 + gn) * cfg.GF],
                            pc[:ssz, :gn * cfg.GF], AF.Copy)

                # --- inner = LU @ (2C) + B ;  y = L @ inner + A ---
                Xin = poolB.tile([128, cfg.nVt * BF], bf16, tag="B")
                ytile = poolC.tile([128, cfg.nVt * BF], bf16, tag="YT")
                for phase in range(2):
                    srcL = LUT[cfg.name] if phase == 0 else LT[cfg.name]
                    nS = cfg.nVsp if phase == 0 else cfg.nVt
                    ssizes = ([cfg.sps(s) for s in range(nS)] if phase == 0
                              else [cfg.vts(s) for s in range(nS)])
                    rhs = XC if phase == 0 else Xin
                    rhs_w = BL * F if phase == 0 else BF
                    Wacc = W_sb[f"B{cfg.name}" if phase == 0 else f"A{cfg.name}"]
                    dst = Xin if phase == 0 else ytile
                    for t in range(cfg.nVt):
                        vsz = cfg.vts(t)
                        for pc0 in range(0, BF, 1024):
                            pw = min(1024, BF - pc0)
                            pi = psbig.tile([128, max(pw, 512)], f32, tag="big")
                            for nk in range(0, pw, 512):
                                n0 = pc0 + nk
                                n1 = min(n0 + 512, pc0 + pw)
                                for s in range(nS):
                                    ssz = ssizes[s]
                                    nc.tensor.matmul(
                                        pi[:vsz, n0 - pc0:n1 - pc0],
                                        srcL[:ssz, s * V + t * 128:
                                             s * V + t * 128 + vsz],
                                        rhs[:ssz, s * rhs_w + n0:
                                            s * rhs_w + n1],
                                        start=(s == 0), stop=False,
                                        skip_group_check=True)
                                for g in range(n0 // cfg.GF,
                                               (n1 + cfg.GF - 1) // cfg.GF):
                                    nc.tensor.matmul(
                                        pi[:vsz, g * cfg.GF - pc0:
                                           (g + 1) * cfg.GF - pc0],
                                        XFrep[:, g * V + t * 128:
                                              g * V + t * 128 + vsz],
                                        Wacc[:],
                                        start=False, stop=True,
                                        skip_group_check=True)
                            if last and phase == 1:
                                # reorder (b,fo) -> (fo,b) for output staging
                                nc.vector.tensor_copy(
                                    dst[:vsz, t * BF + pc0: t * BF + pc0 + pw]
                                    .rearrange("p (c b) -> p c b", b=BL),
                                    pi[:vsz, :pw]
                                    .rearrange("p (b c) -> p c b", c=3))
                            elif phase == 0:
                                nc.scalar.activation(
                                    dst[:vsz, t * BF + pc0: t * BF + pc0 + pw],
                                    pi[:vsz, :pw], AF.Copy)
                            else:
                                nc.vector.tensor_copy(
                                    dst[:vsz, t * BF + pc0: t * BF + pc0 + pw],
                                    pi[:vsz, :pw])

                if not last:
                    # --- back-transpose to packed F-layout of next level ---
                    Gp, nGp = cfg.Gp, cfg.nGp
                    XFn = poolA.tile([128, nGp * V], bf16, tag="A")
                    for t in range(cfg.nVt):
                        vsz = cfg.vts(t)
                        for q0 in range(0, nGp, 4):
                            qn = min(4, nGp - q0)
                            pt = pstr.tile([128, 512], bf16, tag="tr")
                            for qi in range(qn):
                                gp = q0 + qi
                                nc.tensor.transpose(
                                    pt[:, qi * 128: qi * 128 + vsz],
                                    ytile[:vsz, t * BF + gp * 128:
                                          t * BF + (gp + 1) * 128],
                                    ident_b[:vsz, :vsz])
                            dstv = XFn[:].rearrange("p (g v) -> p g v", v=V)
                            nc.scalar.activation(
                                dstv[:, q0:q0 + qn, t * 128:t * 128 + vsz],
                                pt[:].rearrange("p (q v) -> p q v", v=128)
                                [:, :qn, :vsz],
                                AF.Copy)

                    # --- BN stats (bf16 pre-BN values) -> AllReduce -> s,t ---
                    FD = nGp * V
                    nch = (FD + 511) // 512
                    bnst = miscp.tile([128, nch * 6], f32, tag="bnst")
                    for ch in range(nch):
                        c0_, c1_ = ch * 512, min((ch + 1) * 512, FD)
                        nc.vector.bn_stats(
                            bnst[:, ch * 6:(ch + 1) * 6], XFn[:, c0_:c1_])
                    aggr = miscp.tile([128, 2], f32, tag="aggr")
                    nc.vector.bn_aggr(
                        aggr[:], bnst[:].rearrange("p (c s) -> p c s", s=6))
                    part = miscp.tile([128, 2], f32, tag="part")
                    nc.vector.tensor_tensor(
                        out=part[:, 1:2], in0=aggr[:, 0:1], in1=aggr[:, 0:1],
                        op=ALU.mult)
                    nc.vector.tensor_tensor(
                        out=part[:, 1:2], in0=part[:, 1:2], in1=aggr[:, 1:2],
                        op=ALU.add)
                    nc.vector.tensor_scalar_mul(part[:, 1:2], part[:, 1:2],
                                                float(FD))
                    nc.vector.tensor_scalar_mul(part[:, 0:1], aggr[:, 0:1],
                                                float(FD))
                    pst = pslin.tile([128, 512], f32, tag="lin")
                    nc.tensor.matmul(pst[:1, :F], part[:, 0:1], sel_sb[F][:],
                                     start=True, stop=True)
                    nc.tensor.matmul(pst[:1, F:2 * F], part[:, 1:2],
                                     sel_sb[F][:], start=True, stop=True)
                    stats_l = miscp.tile([1, 2 * F], f32, tag="statl")
                    nc.vector.tensor_copy(stats_l[:], pst[:1, :2 * F])
                    bin_ = dramp.tile([1, 2 * F], f32, tag=f"arin{ar_idx}")
                    bout = dramp.tile([1, 2 * F], f32, tag=f"arout{ar_idx}")
                    nc.gpsimd.dma_start(bin_[:], stats_l[:])
                    nc.gpsimd.collective_compute(
                        "AllReduce", ALU.add,
                        replica_groups=[list(range(NCORES))],
                        ins=[bin_.opt()], outs=[bout.opt()])
                    stats_g = miscp.tile([1, 2 * F], f32, tag="statg")
                    nc.sync.dma_start(stats_g[:], bout[:])
                    n_g = float(B * V)
                    # tmp cols [0:F]=mu, [F:2F]=var->rstd ; st cols [0:F]=s, [F:2F]=t
                    st = miscp.tile([1, 2 * F], f32, tag="st")
                    tmp = miscp.tile([1, 2 * F], f32, tag="sttmp")
                    mu2 = miscp.tile([1, F], f32, tag="mu2")
                    nc.vector.tensor_scalar_mul(tmp[:, :2 * F], stats_g[:],
                                                1.0 / n_g)
                    nc.vector.tensor_tensor(out=mu2[:], in0=tmp[:, 0:F],
                                            in1=tmp[:, 0:F], op=ALU.mult)
                    nc.vector.tensor_tensor(out=tmp[:, F:2 * F],
                                            in0=tmp[:, F:2 * F],
                                            in1=mu2[:], op=ALU.subtract)
                    nc.scalar.activation(tmp[:, F:2 * F], tmp[:, F:2 * F],
                                         AF.Sqrt, bias=eps_t[:])
                    nc.vector.reciprocal(tmp[:, F:2 * F], tmp[:, F:2 * F])
                    nc.vector.tensor_tensor(out=st[:, 0:F],
                                            in0=tmp[:, F:2 * F],
                                            in1=gb_sb[li][:, 0:F], op=ALU.mult)
                    nc.vector.tensor_tensor(out=mu2[:], in0=tmp[:, 0:F],
                                            in1=st[:, 0:F], op=ALU.mult)
                    nc.vector.tensor_tensor(out=st[:, F:2 * F],
                                            in0=gb_sb[li][:, F:2 * F],
                                            in1=mu2[:], op=ALU.subtract)
                    pss = pslin.tile([128, 512], f32, tag="lin", name="pss")
                    nc.tensor.transpose(pss[:2 * F, 0:1], st[:],
                                        ident_f[:1, :1])
                    stc = miscp.tile([128, 2], f32, tag=f"stc{ar_idx}")
                    for j in range(Gp):
                        nc.vector.tensor_copy(stc[j * F:(j + 1) * F, 0:1],
                                              pss[:F, 0:1])
                        nc.vector.tensor_copy(stc[j * F:(j + 1) * F, 1:2],
                                              pss[F:2 * F, 0:1])
                    ar_idx += 1
                    nc.scalar.activation(
                        XFn[:], XFn[:], AF.Relu,
                        scale=stc[:, 0:1], bias=stc[:, 1:2])
                    XF_cur = XFn
                else:
                    # --- stage output: ytile [v, fo*32+b] -> [b, v*3+fo] ---
                    for t in range(cfg.nVt):
                        pt = pstr.tile([128, 512], bf16, tag="tr")
                        nc.tensor.transpose(
                            pt[:96, :128],
                            ytile[:128, t * BF:(t + 1) * BF],
                            ident_b[:128, :128])
                        och = outp.tile([BL, 384], f32, tag="out")
                        for fo in range(3):
                            nc.vector.tensor_scalar_add(
                                och[:].rearrange("b (v f) -> b v f", f=3)
                                [:, :, fo],
                                pt[fo * 32:(fo + 1) * 32, :128],
                                float(b3_imm[fo]))
                        nc.sync.dma_start(
                            ydram[:, t * 384:(t + 1) * 384], och[:])

    nc.compile()
    return nc


def kernel(**inputs):
    import sys
    for p in ("/opt/trn_rl_repo", "/opt/trn_rl_repo/concourse"):
        if p not in sys.path:
            sys.path.insert(0, p)
    from concourse.bass_utils import run_bass_kernel_spmd

    host = _build_host(inputs)
    b3 = [float(v) for v in host.pop("b3")]

    key = ("nc",) + tuple(b3)
    if key not in _CACHE:
        _CACHE[key] = _build_nc(b3)
    nc = _CACHE[key]

    in_maps = []
    for c in range(NCORES):
        m = {k: v for k, v in host.items() if k != "xT"}
        m["xT"] = np.ascontiguousarray(host["xT"][:, c * BL:(c + 1) * BL])
        in_maps.append(m)
    res = run_bass_kernel_spmd(nc, in_maps, core_ids=list(range(NCORES)))
    out = np.concatenate(
        [r["y"].reshape(BL, 1280, 3) for r in res.results], axis=0)
    return out.astype(np.float32)


if __name__ == "__main__":
    import reference as R
    inp = R.setup_inputs()
    inp = {k: np.asarray(v) for k, v in inp.items()}
    act = kernel(**inp)
    exp = np.asarray(R.reference(**inp))
    err = np.linalg.norm(act - exp) / np.linalg.norm(exp)
    print("Relative error:", err)



# revision 11
# speedup vs baseline: 1.7948x; 1.7948x over previous
"""Trainium2 Bass kernel for nn_Graph_CNN_Feat_Mesh (Chebyshev GNN decoder).

Strategy (per-core, data-parallel over batch B=256 -> 32/core):
  - All spmms are dense matmuls on the tensor engine (PE) in bf16.
    For K=3 Chebyshev conv:  y = A(x) + L @ B(x) + (2 L^2) @ C(x)
    with A = W0-W2, B = W1, C = W2 applied per-vertex in feature space.
    For up4-preceded layers, replication is folded into the host-side
    matrices:  y = A(x_up) + (L U) @ B(x320) + (2 L^2 U) @ C(x320),
    so both spmms contract over the small pre-upsample vertex space.
  - Layers c0-c2 run the spmm TRANSPOSED (lhsT = feature tiles, rhs = L
    tiles), emitting the next layer's packed F-layout directly: no
    back-transposes.  The A-term accumulates into the same PSUM with a
    stride-0 broadcast rhs for the up4 replication.
  - BatchNorm (training mode, global batch stats) is exact: per-core
    partial sums are AllGather'd across the 8 cores (cheaper than
    AllReduce) and summed locally; scale/shift+relu is applied in column
    chunks feeding the next layer's matmuls incrementally.
  - FC head runs in bf16 with fp32 PSUM accumulation; weight DMAs are
    issued in consumption order and big late-use matrices (L2, 2*L2^2)
    alias the FC weight SBUF space (chunked so the tiny BN collective
    DMAs never queue behind a long transfer).
"""

import numpy as np

B = 256
NCORES = 8
BL = B // NCORES  # 32
EPS = 1e-5

_CACHE = {}


def _split_W(W):
    W = np.asarray(W, np.float32)
    return W[:, 0::3], W[:, 1::3], W[:, 2::3]


def _dense_L(rows, cols, vals, V):
    L = np.zeros((V, V), np.float32)
    np.add.at(L, (np.asarray(rows), np.asarray(cols)), np.asarray(vals, np.float32))
    return L


def _pad_rows(a, m):
    if a.shape[0] % m == 0:
        return a
    p = m - a.shape[0] % m
    return np.concatenate([a, np.zeros((p,) + a.shape[1:], a.dtype)], 0)


def _stiles(a):
    """[U, V] -> [128, nS*V] with s-tiles of 128 source rows side by side."""
    a = _pad_rows(np.ascontiguousarray(a), 128)
    nS = a.shape[0] // 128
    return np.concatenate([a[s * 128:(s + 1) * 128, :] for s in range(nS)], axis=1)


def _wbd(M, G, Fin, Fout):
    """Block-diagonal weight [128, G*Fout]; block j holds M.T ([Fin, Fout])."""
    out = np.zeros((128, G * Fout), np.float32)
    for j in range(G):
        out[j * Fin:(j + 1) * Fin, j * Fout:(j + 1) * Fout] = M.T
    return out


class _LCfg:
    def __init__(self, name, Vin, Vsp, V, Fin, Fout, up4):
        self.name = name
        self.Vin = Vin            # per-g input column span of XF
        self.Vsp = Vsp            # source vertex space of B/C linears
        self.V = V                # output vertex count
        self.Fin = Fin
        self.Fout = Fout
        self.G = 128 // Fin       # input batch packs
        self.nG = BL // self.G
        self.GF = self.G * Fout
        self.Gp = 128 // Fout     # output batch packs
        self.BF = BL * Fout
        self.nGp = self.BF // 128  # output 128-col blocks
        self.nS = (Vsp + 127) // 128
        self.up4 = up4

    def sps(self, s):
        return min(128, self.Vsp - s * 128)


CFG = [
    _LCfg("c0", 80, 80, 320, 64, 64, True),
    _LCfg("c1", 320, 320, 320, 64, 32, False),
    _LCfg("c2", 320, 320, 1280, 32, 32, True),
]
# c3 (V-layout output layer): Fin=32, Fout=3, G=4, V=Vsp=1280


def _build_host(inputs):
    import ml_dtypes
    bf = ml_dtypes.bfloat16
    f32 = np.float32
    d = {}

    # ---- FC head ----
    xT = np.ascontiguousarray(np.asarray(inputs["x"], f32).T)  # [2048, 256]
    d["xTp_full"] = xT  # sliced + packed per core in kernel()
    fc1wT = np.ascontiguousarray(np.asarray(inputs["fc1_w"], f32).T)  # [2048, 512]
    d["fc1w"] = np.ascontiguousarray(
        fc1wT.reshape(16, 128, 512).transpose(1, 0, 2).reshape(128, 16 * 512)
    ).astype(bf)
    fc2wT = np.ascontiguousarray(np.asarray(inputs["fc2_w"], f32).T)  # [512, 5120]
    f2 = fc2wT.reshape(4, 128, 5120)
    for mc in range(4):
        d[f"fc2w{mc}"] = np.ascontiguousarray(
            f2[:, :, mc * 1280:(mc + 1) * 1280].transpose(1, 0, 2).reshape(128, 4 * 1280)
        ).astype(bf)
    smalls = np.zeros((128, 100), f32)
    for j in range(2):
        smalls[j * 64:(j + 1) * 64, 0:64] += np.eye(64, dtype=f32)
    for j in range(4):
        smalls[j * 32:(j + 1) * 32, 64:96] += np.eye(32, dtype=f32)
    smalls[:, 96:100] = np.asarray(inputs["fc1_b"], f32).reshape(4, 128).T
    d["smalls"] = smalls

    # ---- L matrices ----
    L1 = _dense_L(inputs["L1_rows"], inputs["L1_cols"], inputs["L1_vals"], 320)
    L2 = _dense_L(inputs["L2_rows"], inputs["L2_cols"], inputs["L2_vals"], 1280)
    U1 = np.repeat(np.eye(80, dtype=f32), 4, axis=0)    # [320, 80]
    U2 = np.repeat(np.eye(320, dtype=f32), 4, axis=0)   # [1280, 320]
    LU0 = (L1 @ U1).T                                   # [80, 320]
    LLU0 = 2.0 * (L1 @ (L1 @ U1)).T
    d["LU0p"] = _pad_rows(np.concatenate([LU0, LLU0], axis=1), 128).astype(bf)
    LT1 = _stiles(L1.T)                                 # [128, 3*320]
    LL1 = _stiles(2.0 * (L1 @ L1).T)
    d["LT1p"] = np.concatenate([LT1, LL1], axis=1).astype(bf)
    LU2 = _stiles((L2 @ U2).T)                          # [128, 3*1280]
    LLU2 = _stiles(2.0 * (L2 @ (L2 @ U2)).T)
    d["LU2p"] = np.concatenate([LU2, LLU2], axis=1).astype(bf)
    d["LT2"] = _stiles(L2.T).astype(bf)                 # [128, 10*1280]
    d["LL2"] = _stiles(2.0 * (L2 @ L2).T).astype(bf)

    # ---- Chebyshev linear weight blocks ----
    blks = []
    offs = {}

    def add(nm, arr):
        offs[nm] = sum(b.shape[1] for b in blks)
        blks.append(arr)

    for li, (cfg, wn) in enumerate(zip(CFG, ["cl0_w", "cl1_w", "cl2_w"])):
        W0, W1, W2 = _split_W(inputs[wn])
        A = W0 - W2
        add(f"B{li}", _wbd(W1, cfg.G, cfg.Fin, cfg.Fout))
        add(f"C{li}", _wbd(W2, cfg.G, cfg.Fin, cfg.Fout))
        if cfg.name == "c1":
            for dl in range(2):
                M = np.zeros((128, 128), f32)
                for j in range(2):
                    M[j * 64:(j + 1) * 64,
                      (2 * dl + j) * 32:(2 * dl + j + 1) * 32] = A.T
                add(f"A1_{dl}", M)
        else:
            add(f"A{li}", _wbd(A, cfg.G, cfg.Fin, cfg.Fout))
    W0, W1, W2 = _split_W(inputs["cl3_w"])
    add("B3", _wbd(W1, 4, 32, 3))
    add("C3", _wbd(W2, 4, 32, 3))
    add("A3", _wbd(W0 - W2, 4, 32, 3))
    d["wblk"] = np.concatenate(blks, axis=1).astype(bf)
    d["_woffs"] = offs  # not uploaded

    for i, (g, b) in enumerate([("bn0_g", "bn0_b"), ("bn1_g", "bn1_b"),
                                ("bn2_g", "bn2_b")]):
        gb = np.concatenate([np.asarray(inputs[g], f32),
                             np.asarray(inputs[b], f32)])
        d[f"gb{i}"] = np.ascontiguousarray(gb[None, :])  # [1, 2F]
    b3 = np.asarray(inputs["cl3_b"], f32)
    d["b3r"] = np.ascontiguousarray(np.tile(b3, BL)[None, :])  # [1, 96]
    return d


def _build_nc(woffs):
    import sys
    for p in ("/opt/trn_rl_repo", "/opt/trn_rl_repo/concourse"):
        if p not in sys.path:
            sys.path.insert(0, p)
    import concourse.bass as bass  # noqa
    import concourse.mybir as mybir
    import concourse.tile as tile
    from concourse import bacc
    from concourse.masks import make_identity

    f32 = mybir.dt.float32
    bf16 = mybir.dt.bfloat16
    AF = mybir.ActivationFunctionType
    ALU = mybir.AluOpType

    nc = bacc.Bacc(None, target_bir_lowering=False)

    xTp = nc.dram_tensor("xTp", [128, 16 * BL], bf16, kind="ExternalInput")
    smalls_d = nc.dram_tensor("smalls", [128, 100], f32, kind="ExternalInput")
    fc1w_d = nc.dram_tensor("fc1w", [128, 16 * 512], bf16, kind="ExternalInput")
    fc2w_d = [nc.dram_tensor(f"fc2w{mc}", [128, 4 * 1280], bf16,
                             kind="ExternalInput") for mc in range(4)]
    wblk_d = nc.dram_tensor("wblk", [128, 1188], bf16, kind="ExternalInput")
    LU0p_d = nc.dram_tensor("LU0p", [128, 640], bf16, kind="ExternalInput")
    LT1p_d = nc.dram_tensor("LT1p", [128, 1920], bf16, kind="ExternalInput")
    LU2p_d = nc.dram_tensor("LU2p", [128, 7680], bf16, kind="ExternalInput")
    LT2_d = nc.dram_tensor("LT2", [128, 12800], bf16, kind="ExternalInput")
    LL2_d = nc.dram_tensor("LL2", [128, 12800], bf16, kind="ExternalInput")
    gbs_d = [nc.dram_tensor(f"gb{i}", [1, 2 * F], f32, kind="ExternalInput")
             for i, F in enumerate([64, 32, 32])]
    b3r_d = nc.dram_tensor("b3r", [1, 96], f32, kind="ExternalInput")
    ydram = nc.dram_tensor("y", [128, 960], f32, kind="ExternalOutput")

    with tile.TileContext(nc) as tc:
        with (
            tc.tile_pool(name="wpool", bufs=1) as wpool,
            tc.tile_pool(name="actp", bufs=1) as actp,
            tc.tile_pool(name="misc", bufs=1) as miscp,
            tc.tile_pool(name="pslin", bufs=2, space="PSUM") as pslin,
            tc.tile_pool(name="psW", bufs=3, space="PSUM") as psW,
            tc.tile_pool(name="dram", bufs=1, space="DRAM") as dramp,
        ):
            # ================= SBUF tiles =================
            W1 = wpool.tile([128, 20480], bf16, tag="W1")      # fc2w
            W2 = wpool.tile([128, 8192], bf16, tag="W2")       # fc1w
            LU2p = wpool.tile([128, 7680], bf16, tag="LU2p2")
            LT2 = wpool.tile([128, 12800], bf16, tag="LT2")
            LL2 = wpool.tile([128, 12800], bf16, tag="LL2")
            wblk = wpool.tile([128, 1188], bf16, tag="wblk")
            LU0p = wpool.tile([128, 640], bf16, tag="LU0p")
            LT1p = wpool.tile([128, 1920], bf16, tag="LT1p")
            smalls = wpool.tile([128, 100], f32, tag="smalls")
            xT = wpool.tile([128, 16 * BL], bf16, tag="xT")
            gb_sb = [wpool.tile([1, 2 * F], f32, tag=f"gb{i}",
                                name=f"gb{i}")
                     for i, F in enumerate([64, 32, 32])]
            b3r = wpool.tile([1, 96], f32, tag="b3r")

            def WB(nm, w):
                return wblk[:, woffs[nm]:woffs[nm] + w]

            # ---- DMA issue order == consumption order (SP queue) ----
            nc.sync.dma_start(xT[:], xTp[:])
            nc.sync.dma_start(smalls[:], smalls_d[:])
            nc.sync.dma_start(W2[:, 0:8192], fc1w_d[:])
            for mc in range(4):
                nc.sync.dma_start(W1[:, mc * 5120:(mc + 1) * 5120], fc2w_d[mc][:])
            nc.sync.dma_start(wblk[:], wblk_d[:])
            nc.sync.dma_start(LU0p[:], LU0p_d[:])
            nc.sync.dma_start(LT1p[:], LT1p_d[:])
            for i in range(3):
                nc.sync.dma_start(gb_sb[i][:], gbs_d[i][:])
            nc.sync.dma_start(b3r[:], b3r_d[:])
            # late-use loads, chunked so the tiny BN-collective DMAs never
            # wait long on the DMA_ENGINES queue
            for k in range(4):
                nc.sync.dma_start(LU2p[:, k * 1920:(k + 1) * 1920],
                                  LU2p_d[:, k * 1920:(k + 1) * 1920])
            for k in range(8):
                nc.sync.dma_start(LT2[:, k * 1600:(k + 1) * 1600],
                                  LT2_d[:, k * 1600:(k + 1) * 1600])
            for k in range(8):
                nc.sync.dma_start(LL2[:, k * 1600:(k + 1) * 1600],
                                  LL2_d[:, k * 1600:(k + 1) * 1600])

            # ---- constants / act-table warm ----
            eps_t = miscp.tile([1, 1], f32, tag="eps")
            nc.gpsimd.memset(eps_t[:], EPS)
            ones8 = miscp.tile([8, 1], f32, tag="ones8")
            nc.gpsimd.memset(ones8[:], 1.0)
            onesr = miscp.tile([1, 128], bf16, tag="onesr")
            nc.gpsimd.memset(onesr[:], 1.0)
            b3rb = miscp.tile([1, 96], bf16, tag="b3rb")
            nc.vector.tensor_copy(b3rb[:], b3r[:])
            ident_f = miscp.tile([128, 128], f32, tag="identf")
            make_identity(nc, ident_f[:])
            warm = miscp.tile([1, 4], f32, tag="warm")
            nc.gpsimd.memset(warm[:], 1.0)
            nc.scalar.activation(warm[:, 0:1], warm[:, 0:1], AF.Relu)
            nc.scalar.activation(warm[:, 1:2], warm[:, 1:2], AF.Copy)
            nc.scalar.activation(warm[:, 2:3], warm[:, 2:3], AF.Sqrt)

            # round-robin copy engines for PSUM->SBUF evacuation
            cp_state = [0]

            def cp(dst, src):
                e = cp_state[0] % 2
                cp_state[0] += 1
                if e == 0:
                    nc.scalar.activation(dst, src, AF.Copy)
                else:
                    nc.vector.tensor_copy(dst, src)

            # ================= FC head (bf16) =================
            h1T = miscp.tile([128, 4 * BL], bf16, tag="h1T")
            ps1 = pslin.tile([128, 512], f32, tag="lin")
            for mt in range(4):
                for kt in range(16):
                    nc.tensor.matmul(
                        ps1[:, mt * BL:(mt + 1) * BL],
                        W2[:, kt * 512 + mt * 128: kt * 512 + (mt + 1) * 128],
                        xT[:, kt * BL:(kt + 1) * BL],
                        start=(kt == 0), stop=(kt == 15),
                        skip_group_check=True)
                nc.scalar.activation(
                    h1T[:, mt * BL:(mt + 1) * BL], ps1[:, mt * BL:(mt + 1) * BL],
                    AF.Relu, bias=smalls[:, 96 + mt:97 + mt])

            XF0 = actp.tile([128, 16 * 80], bf16, tag="XF0")
            for mc in range(4):
                ps2 = psW.tile([128, 1024], f32, tag="big")
                for mi in range(10):
                    for kt in range(4):
                        nc.tensor.matmul(
                            ps2[:, mi * BL:(mi + 1) * BL],
                            W1[:, mc * 5120 + kt * 1280 + mi * 128:
                               mc * 5120 + kt * 1280 + (mi + 1) * 128],
                            h1T[:, kt * BL:(kt + 1) * BL],
                            start=(kt == 0), stop=(kt == 3),
                            skip_group_check=True)
                # psum [(v0%2)*64+f, b] -> XF0[(b%2)*64+f, (b//2)*80 + v0]
                src4 = ps2[:, 0:320].rearrange("p (i g j) -> p i g j", g=16, j=2)
                dst4 = XF0[:].rearrange("p (g u q) -> p g u q", u=40, q=2)
                for p0 in range(2):
                    for j in range(2):
                        nc.scalar.activation(
                            dst4[j * 64:(j + 1) * 64, :,
                                 mc * 10:(mc + 1) * 10, p0]
                            .rearrange("p g i -> p i g"),
                            src4[p0 * 64:(p0 + 1) * 64, :, :, j],
                            AF.Copy)

            # ================= cheby layers c0-c2 (F-layout) =================
            XF_cur = XF0

            for li, cfg in enumerate(CFG):
                V, Vin, F = cfg.V, cfg.Vin, cfg.Fout
                BF, nS, nGp = cfg.BF, cfg.nS, cfg.nGp
                # L-matrix rhs tiles: [128, nS*V (L-part) | nS*V (2L^2-part)]
                if cfg.name == "c0":
                    Lr, lw = LU0p, 320
                elif cfg.name == "c1":
                    Lr, lw = LT1p, 320
                else:
                    Lr, lw = LU2p, 1280

                # ---- B/C linears into source-vertex space ----
                XB = actp.tile([128, 3072], bf16, tag="XB",
                               name="XB")[:, :nS * BF]
                XC = actp.tile([128, 3072], bf16, tag="XC",
                               name="XC")[:, :nS * BF]
                gpack = max(1, 512 // cfg.GF)
                for s in range(nS):
                    ssz = cfg.sps(s)
                    for dst, wnm in ((XB, f"B{li}"), (XC, f"C{li}")):
                        Wt = WB(wnm, cfg.GF)
                        for g0 in range(0, cfg.nG, gpack):
                            gn = min(gpack, cfg.nG - g0)
                            pc = pslin.tile([128, 512], f32, tag="lin")
                            for gi in range(gn):
                                g = g0 + gi
                                nc.tensor.matmul(
                                    pc[:ssz, gi * cfg.GF:(gi + 1) * cfg.GF],
                                    XF_cur[:, g * Vin + s * 128:
                                           g * Vin + s * 128 + ssz],
                                    Wt, start=True, stop=True,
                                    skip_group_check=True)
                            cp(dst[:ssz, s * BF + g0 * cfg.GF:
                                   s * BF + (g0 + gn) * cfg.GF],
                               pc[:ssz, :gn * cfg.GF])

                # ---- transposed spmm + copies + 512-wide stats windows ----
                XFn = actp.tile([128, nGp * V], bf16, tag=f"XFn{li}")
                FD = nGp * V
                nch = FD // 512
                bnst = miscp.tile([128, nch * 6], f32, tag=f"bnst{li}")
                stat_done = [0, 0]  # cols copied, windows emitted

                def emit_stats(done, XFn=XFn, bnst=bnst, sd=stat_done, nch=nch):
                    sd[0] = done
                    while sd[1] < nch and (sd[1] + 1) * 512 <= sd[0]:
                        ci = sd[1]
                        nc.vector.bn_stats(
                            bnst[:, ci * 6:(ci + 1) * 6],
                            XFn[:, ci * 512:(ci + 1) * 512])
                        sd[1] += 1

                def spmm_group(ps, pbase, gp, w0, wcw):
                    """Accumulate output block (gp, w0:w0+wcw) into ps cols
                    pbase:pbase+wcw (wcw <= 512)."""
                    for half, XS in ((0, XB), (1, XC)):
                        for s in range(nS):
                            ssz = cfg.sps(s)
                            nc.tensor.matmul(
                                ps[:, pbase:pbase + wcw],
                                XS[:ssz, s * BF + gp * 128:
                                   s * BF + (gp + 1) * 128],
                                Lr[:ssz, half * nS * lw + s * lw + w0:
                                   half * nS * lw + s * lw + w0 + wcw],
                                start=(half == 0 and s == 0), stop=False,
                                skip_group_check=True)
                    if cfg.name == "c1":
                        for dl in range(2):
                            nc.tensor.matmul(
                                ps[:, pbase:pbase + wcw],
                                WB(f"A1_{dl}", 128),
                                XF_cur[:, (2 * gp + dl) * Vin + w0:
                                       (2 * gp + dl) * Vin + w0 + wcw],
                                start=False, stop=(dl == 1),
                                skip_group_check=True)
                    else:
                        rhs = XF_cur[:, gp * Vin + w0 // 4:
                                     gp * Vin + w0 // 4 + wcw // 4]
                        rhs = rhs.broadcast_to([128, wcw // 4, 4])
                        nc.tensor.matmul(
                            ps[:, pbase:pbase + wcw], WB(f"A{li}", 128), rhs,
                            start=False, stop=True, skip_group_check=True)

                if V <= 512:
                    # pack 2 gp-blocks per PSUM tile (bank-aligned at 512)
                    for gpp in range(0, nGp, 2):
                        ps = psW.tile([128, 1024], f32, tag="big")
                        for gi in range(2):
                            spmm_group(ps, gi * 512, gpp + gi, 0, V)
                        cp(XFn[:, gpp * V:(gpp + 2) * V]
                           .rearrange("p (u w) -> p u w", w=V),
                           ps[:].rearrange("p (u w) -> p u w", w=512)[:, :, :V])
                        emit_stats((gpp + 2) * V)
                else:
                    # c2: V=1280 per gp -> tiles of 1024 + 256
                    for gp in range(nGp):
                        for w0 in (0, 1024):
                            wcw = min(1024, V - w0)
                            ps = psW.tile([128, 1024], f32, tag="big")
                            for nk in range(0, wcw, 512):
                                sub = min(512, wcw - nk)
                                spmm_group(ps, nk, gp, w0 + nk, sub)
                            cp(XFn[:, gp * V + w0: gp * V + w0 + wcw],
                               ps[:, :wcw])
                            emit_stats(gp * V + w0 + wcw)

                # ---- BN: partial sums -> AllGather -> scale/shift ----
                n_g = float(B * V)
                aggr = miscp.tile([128, 2], f32, tag="aggr")
                nc.vector.bn_aggr(
                    aggr[:], bnst[:].rearrange("p (c s) -> p c s", s=6))
                part = miscp.tile([128, 2], f32, tag="part")
                nc.vector.tensor_tensor(
                    out=part[:, 1:2], in0=aggr[:, 0:1], in1=aggr[:, 0:1],
                    op=ALU.mult)
                nc.vector.tensor_tensor(
                    out=part[:, 1:2], in0=part[:, 1:2], in1=aggr[:, 1:2],
                    op=ALU.add)
                nc.vector.tensor_scalar_mul(part[:, 1:2], part[:, 1:2],
                                            float(FD))
                nc.vector.tensor_scalar_mul(part[:, 0:1], aggr[:, 0:1],
                                            float(FD))
                sel = smalls[:, 0:64] if F == 64 else smalls[:, 64:96]
                pst = pslin.tile([128, 512], f32, tag="lin")
                nc.tensor.matmul(pst[:1, :F], part[:, 0:1], sel,
                                 start=True, stop=True, skip_group_check=True)
                nc.tensor.matmul(pst[:1, F:2 * F], part[:, 1:2], sel,
                                 start=True, stop=True, skip_group_check=True)
                stats_l = miscp.tile([1, 2 * F], f32, tag="statl")
                nc.vector.tensor_copy(stats_l[:], pst[:1, :2 * F])
                bin_ = dramp.tile([1, 2 * F], f32, tag=f"arin{li}")
                bout = dramp.tile([NCORES, 2 * F], f32, tag=f"arout{li}")
                nc.gpsimd.dma_start(bin_[:], stats_l[:])
                nc.gpsimd.collective_compute(
                    "AllGather", ALU.bypass,
                    replica_groups=[list(range(NCORES))],
                    ins=[bin_.opt()], outs=[bout.opt()])
                statg8 = miscp.tile([NCORES, 2 * F], f32, tag="statg8")
                nc.gpsimd.dma_start(statg8[:], bout[:])
                psg = pslin.tile([128, 512], f32, tag="lin")
                nc.tensor.matmul(psg[:1, :2 * F], ones8[:], statg8[:],
                                 start=True, stop=True, skip_group_check=True)
                # s,t from global sums
                st = miscp.tile([1, 2 * F], f32, tag="st")
                tmp = miscp.tile([1, 2 * F], f32, tag="sttmp")
                mu2 = miscp.tile([1, F], f32, tag="mu2")
                nc.vector.tensor_scalar_mul(tmp[:, :2 * F], psg[:1, :2 * F],
                                            1.0 / n_g)
                nc.vector.tensor_tensor(out=mu2[:], in0=tmp[:, 0:F],
                                        in1=tmp[:, 0:F], op=ALU.mult)
                nc.vector.tensor_tensor(out=tmp[:, F:2 * F],
                                        in0=tmp[:, F:2 * F],
                                        in1=mu2[:], op=ALU.subtract)
                nc.scalar.activation(tmp[:, F:2 * F], tmp[:, F:2 * F],
                                     AF.Sqrt, bias=eps_t[:])
                nc.vector.reciprocal(tmp[:, F:2 * F], tmp[:, F:2 * F])
                nc.vector.tensor_tensor(out=st[:, 0:F],
                                        in0=tmp[:, F:2 * F],
                                        in1=gb_sb[li][:, 0:F], op=ALU.mult)
                nc.vector.tensor_tensor(out=mu2[:], in0=tmp[:, 0:F],
                                        in1=st[:, 0:F], op=ALU.mult)
                nc.vector.tensor_tensor(out=st[:, F:2 * F],
                                        in0=gb_sb[li][:, F:2 * F],
                                        in1=mu2[:], op=ALU.subtract)
                pss = pslin.tile([128, 512], f32, tag="lin", name="pss")
                nc.tensor.transpose(pss[:2 * F, 0:1], st[:],
                                    ident_f[:1, :1])
                stc = miscp.tile([128, 2], f32, tag=f"stc{li}")
                for j in range(cfg.Gp):
                    nc.vector.tensor_copy(stc[j * F:(j + 1) * F, 0:1],
                                          pss[:F, 0:1])
                    nc.vector.tensor_copy(stc[j * F:(j + 1) * F, 1:2],
                                          pss[F:2 * F, 0:1])
                # chunked scale/shift + relu (Act), g-aligned chunks
                nrc = 4 if FD <= 5120 else 8
                csz = FD // nrc
                for rc in range(nrc):
                    sl = slice(rc * csz, (rc + 1) * csz)
                    nc.scalar.activation(
                        XFn[:, sl], XFn[:, sl], AF.Relu,
                        scale=stc[:, 0:1], bias=stc[:, 1:2])
                XF_cur = XFn

            # ================= c3 (V-layout) + output =================
            # XF_cur = XF2 [128 (j4,c32), 8*1280], G=4, nG=8
            XB3 = actp.tile([128, 3072], bf16, tag="XB",
                            name="XB3")[:, :960]
            XC3 = actp.tile([128, 3072], bf16, tag="XC",
                            name="XC3")[:, :960]
            for s in range(10):
                pc = pslin.tile([128, 512], f32, tag="lin")
                for dst, wnm, off in ((XB3, "B3", 0), (XC3, "C3", 96)):
                    Wt = WB(wnm, 12)
                    for g in range(8):
                        nc.tensor.matmul(
                            pc[:, off + g * 12: off + (g + 1) * 12],
                            XF_cur[:, g * 1280 + s * 128:
                                   g * 1280 + (s + 1) * 128],
                            Wt, start=True, stop=True, skip_group_check=True)
                cp(XB3[:, s * 96:(s + 1) * 96], pc[:, 0:96])
                cp(XC3[:, s * 96:(s + 1) * 96], pc[:, 96:192])

            ysb = miscp.tile([128, 960], f32, tag="ysb")
            for t in range(10):
                pv = psW.tile([128, 1024], f32, tag="big")
                for s in range(10):
                    nc.tensor.matmul(
                        pv[:, 0:96],
                        LT2[:, s * 1280 + t * 128:s * 1280 + (t + 1) * 128],
                        XB3[:, s * 96:(s + 1) * 96],
                        start=(s == 0), stop=False, skip_group_check=True)
                for s in range(10):
                    nc.tensor.matmul(
                        pv[:, 0:96],
                        LL2[:, s * 1280 + t * 128:s * 1280 + (t + 1) * 128],
                        XC3[:, s * 96:(s + 1) * 96],
                        start=False, stop=False, skip_group_check=True)
                for g in range(8):
                    nc.tensor.matmul(
                        pv[:, g * 12:(g + 1) * 12],
                        XF_cur[:, g * 1280 + t * 128:g * 1280 + (t + 1) * 128],
                        WB("A3", 12),
                        start=False, stop=False, skip_group_check=True)
                nc.tensor.matmul(
                    pv[:, 0:96], onesr[:, :128], b3rb[:],
                    start=False, stop=True, skip_group_check=True)
                cp(ysb[:, t * 96:(t + 1) * 96], pv[:, 0:96])
            nc.sync.dma_start(ydram[:, 0:480], ysb[:, 0:480])
            nc.sync.dma_start(ydram[:, 480:960], ysb[:, 480:960])

    nc.compile()
    return nc


def kernel(**inputs):
    import sys
    for p in ("/opt/trn_rl_repo", "/opt/trn_rl_repo/concourse"):
        if p not in sys.path:
            sys.path.insert(0, p)
    from concourse.bass_utils import run_bass_kernel_spmd
    import ml_dtypes

    host = _build_host(inputs)
    woffs = host.pop("_woffs")
    xT_full = host.pop("xTp_full")

    key = ("nc",)
    if key not in _CACHE:
        _CACHE[key] = _build_nc(woffs)
    nc = _CACHE[key]

    in_maps = []
    for c in range(NCORES):
        m = dict(host)
        xc = xT_full[:, c * BL:(c + 1) * BL]  # [2048, 32]
        m["xTp"] = np.ascontiguousarray(
            xc.reshape(16, 128, BL).transpose(1, 0, 2).reshape(128, 16 * BL)
        ).astype(ml_dtypes.bfloat16)
        in_maps.append(m)
    res = run_bass_kernel_spmd(nc, in_maps, core_ids=list(range(NCORES)))
    outs = []
    for c in range(NCORES):
        y = res.results[c]["y"].astype(np.float32)  # [128, 960]
        outs.append(y.reshape(128, 10, BL, 3).transpose(2, 1, 0, 3)
                    .reshape(BL, 1280, 3))
    return np.concatenate(outs, axis=0)


if __name__ == "__main__":
    import reference as R
    inp = R.setup_inputs()
    inp = {k: np.asarray(v) for k, v in inp.items()}
    act = kernel(**inp)
    exp = np.asarray(R.reference(**inp))
    err = np.linalg.norm(act - exp) / np.linalg.norm(exp)
    print("Relative error:", err)


# revision 17
# speedup vs baseline: 1.8381x; 1.0241x over previous
"""Trainium2 Bass kernel for nn_Graph_CNN_Feat_Mesh (Chebyshev GNN decoder).

Strategy (per-core, data-parallel over batch B=256 -> 32/core):
  - All spmms are dense matmuls on the tensor engine (PE) in bf16.
    For K=3 Chebyshev conv:  y = A(x) + L @ B(x) + (2 L^2) @ C(x)
    with A = W0-W2, B = W1, C = W2 applied per-vertex in feature space.
    For up4-preceded layers, replication is folded into the host-side
    matrices:  y = A(x_up) + (L U) @ B(x320) + (2 L^2 U) @ C(x320),
    so both spmms contract over the small pre-upsample vertex space.
  - Layers c0-c2 run the spmm TRANSPOSED (lhsT = feature tiles, rhs = L
    tiles), emitting the next layer's packed F-layout directly: no
    back-transposes.  The A-term accumulates into the same PSUM with a
    stride-0 broadcast rhs for the up4 replication.
  - BatchNorm (training mode, global batch stats) is exact: per-core
    partial sums are AllGather'd across the 8 cores (cheaper than
    AllReduce) and summed locally; scale/shift+relu is applied in column
    chunks feeding the next layer's matmuls incrementally.
  - FC head runs in bf16 with fp32 PSUM accumulation; weight DMAs are
    issued in consumption order and big late-use matrices (L2, 2*L2^2)
    alias the FC weight SBUF space (chunked so the tiny BN collective
    DMAs never queue behind a long transfer).
"""

import numpy as np

B = 256
NCORES = 8
BL = B // NCORES  # 32
EPS = 1e-5

_CACHE = {}


def _split_W(W):
    W = np.asarray(W, np.float32)
    return W[:, 0::3], W[:, 1::3], W[:, 2::3]


def _dense_L(rows, cols, vals, V):
    L = np.zeros((V, V), np.float32)
    np.add.at(L, (np.asarray(rows), np.asarray(cols)), np.asarray(vals, np.float32))
    return L


def _pad_rows(a, m):
    if a.shape[0] % m == 0:
        return a
    p = m - a.shape[0] % m
    return np.concatenate([a, np.zeros((p,) + a.shape[1:], a.dtype)], 0)


def _stiles(a):
    """[U, V] -> [128, nS*V] with s-tiles of 128 source rows side by side."""
    a = _pad_rows(np.ascontiguousarray(a), 128)
    nS = a.shape[0] // 128
    return np.concatenate([a[s * 128:(s + 1) * 128, :] for s in range(nS)], axis=1)


def _wbd(M, G, Fin, Fout):
    """Block-diagonal weight [128, G*Fout]; block j holds M.T ([Fin, Fout])."""
    out = np.zeros((128, G * Fout), np.float32)
    for j in range(G):
        out[j * Fin:(j + 1) * Fin, j * Fout:(j + 1) * Fout] = M.T
    return out


class _LCfg:
    def __init__(self, name, Vin, Vsp, V, Fin, Fout, up4):
        self.name = name
        self.Vin = Vin            # per-g input column span of XF
        self.Vsp = Vsp            # source vertex space of B/C linears
        self.V = V                # output vertex count
        self.Fin = Fin
        self.Fout = Fout
        self.G = 128 // Fin       # input batch packs
        self.nG = BL // self.G
        self.GF = self.G * Fout
        self.Gp = 128 // Fout     # output batch packs
        self.BF = BL * Fout
        self.nGp = self.BF // 128  # output 128-col blocks
        self.nS = (Vsp + 127) // 128
        self.up4 = up4

    def sps(self, s):
        return min(128, self.Vsp - s * 128)


CFG = [
    _LCfg("c0", 80, 80, 320, 64, 64, True),
    _LCfg("c1", 320, 320, 320, 64, 32, False),
    _LCfg("c2", 320, 320, 1280, 32, 32, True),
]
# c3 (V-layout output layer): Fin=32, Fout=3, G=4, V=Vsp=1280


def _build_host(inputs):
    import ml_dtypes
    bf = ml_dtypes.bfloat16
    f32 = np.float32
    d = {}

    # ---- FC head ----
    xT = np.ascontiguousarray(np.asarray(inputs["x"], f32).T)  # [2048, 256]
    d["xTp_full"] = xT  # sliced + packed per core in kernel()
    fc1wT = np.ascontiguousarray(np.asarray(inputs["fc1_w"], f32).T)  # [2048, 512]
    d["fc1w"] = np.ascontiguousarray(
        fc1wT.reshape(16, 128, 512).transpose(1, 0, 2).reshape(128, 16 * 512)
    ).astype(bf)
    fc2wT = np.ascontiguousarray(np.asarray(inputs["fc2_w"], f32).T)  # [512, 5120]
    f2 = fc2wT.reshape(4, 128, 5120)
    for mc in range(4):
        d[f"fc2w{mc}"] = np.ascontiguousarray(
            f2[:, :, mc * 1280:(mc + 1) * 1280].transpose(1, 0, 2).reshape(128, 4 * 1280)
        ).astype(bf)
    smalls = np.zeros((128, 100), f32)
    for j in range(2):
        smalls[j * 64:(j + 1) * 64, 0:64] += np.eye(64, dtype=f32)
    for j in range(4):
        smalls[j * 32:(j + 1) * 32, 64:96] += np.eye(32, dtype=f32)
    smalls[:, 96:100] = np.asarray(inputs["fc1_b"], f32).reshape(4, 128).T
    d["smalls"] = smalls

    # ---- L matrices ----
    L1 = _dense_L(inputs["L1_rows"], inputs["L1_cols"], inputs["L1_vals"], 320)
    L2 = _dense_L(inputs["L2_rows"], inputs["L2_cols"], inputs["L2_vals"], 1280)
    U1 = np.repeat(np.eye(80, dtype=f32), 4, axis=0)    # [320, 80]
    U2 = np.repeat(np.eye(320, dtype=f32), 4, axis=0)   # [1280, 320]
    LU0 = (L1 @ U1).T                                   # [80, 320]
    LLU0 = 2.0 * (L1 @ (L1 @ U1)).T
    d["LU0p"] = _pad_rows(np.concatenate([LU0, LLU0], axis=1), 128).astype(bf)
    LT1 = _stiles(L1.T)                                 # [128, 3*320]
    LL1 = _stiles(2.0 * (L1 @ L1).T)
    d["LT1p"] = np.concatenate([LT1, LL1], axis=1).astype(bf)
    LU2 = _stiles((L2 @ U2).T)                          # [128, 3*1280]
    LLU2 = _stiles(2.0 * (L2 @ (L2 @ U2)).T)
    d["LU2p"] = np.concatenate([LU2, LLU2], axis=1).astype(bf)
    d["LT2"] = _stiles(L2.T).astype(bf)                 # [128, 10*1280]
    d["LL2"] = _stiles(2.0 * (L2 @ L2).T).astype(bf)

    # ---- Chebyshev linear weight blocks ----
    blks = []
    offs = {}

    def add(nm, arr):
        offs[nm] = sum(b.shape[1] for b in blks)
        blks.append(arr)

    for li, (cfg, wn) in enumerate(zip(CFG, ["cl0_w", "cl1_w", "cl2_w"])):
        W0, W1, W2 = _split_W(inputs[wn])
        A = W0 - W2
        add(f"B{li}", _wbd(W1, cfg.G, cfg.Fin, cfg.Fout))
        add(f"C{li}", _wbd(W2, cfg.G, cfg.Fin, cfg.Fout))
        if cfg.name == "c1":
            for dl in range(2):
                M = np.zeros((128, 128), f32)
                for j in range(2):
                    M[j * 64:(j + 1) * 64,
                      (2 * dl + j) * 32:(2 * dl + j + 1) * 32] = A.T
                add(f"A1_{dl}", M)
        else:
            add(f"A{li}", _wbd(A, cfg.G, cfg.Fin, cfg.Fout))
    W0, W1, W2 = _split_W(inputs["cl3_w"])
    add("B3", _wbd(W1, 4, 32, 3))
    add("C3", _wbd(W2, 4, 32, 3))
    add("A3", _wbd(W0 - W2, 4, 32, 3))
    d["wblk"] = np.concatenate(blks, axis=1).astype(bf)
    d["_woffs"] = offs  # not uploaded

    for i, (g, b) in enumerate([("bn0_g", "bn0_b"), ("bn1_g", "bn1_b"),
                                ("bn2_g", "bn2_b")]):
        gb = np.concatenate([np.asarray(inputs[g], f32),
                             np.asarray(inputs[b], f32)])
        d[f"gb{i}"] = np.ascontiguousarray(gb[None, :])  # [1, 2F]
    b3 = np.asarray(inputs["cl3_b"], f32)
    d["b3r"] = np.ascontiguousarray(np.tile(b3, 160)[None, :])  # [1, 480]
    return d


def _build_nc(woffs):
    import sys
    for p in ("/opt/trn_rl_repo", "/opt/trn_rl_repo/concourse"):
        if p not in sys.path:
            sys.path.insert(0, p)
    import concourse.bass as bass  # noqa
    import concourse.mybir as mybir
    import concourse.tile as tile
    from concourse import bacc
    from concourse.masks import make_identity

    f32 = mybir.dt.float32
    bf16 = mybir.dt.bfloat16
    AF = mybir.ActivationFunctionType
    ALU = mybir.AluOpType

    nc = bacc.Bacc(None, target_bir_lowering=False)

    xTp = nc.dram_tensor("xTp", [128, 16 * BL], bf16, kind="ExternalInput")
    smalls_d = nc.dram_tensor("smalls", [128, 100], f32, kind="ExternalInput")
    fc1w_d = nc.dram_tensor("fc1w", [128, 16 * 512], bf16, kind="ExternalInput")
    fc2w_d = [nc.dram_tensor(f"fc2w{mc}", [128, 4 * 1280], bf16,
                             kind="ExternalInput") for mc in range(4)]
    wblk_d = nc.dram_tensor("wblk", [128, 1188], bf16, kind="ExternalInput")
    LU0p_d = nc.dram_tensor("LU0p", [128, 640], bf16, kind="ExternalInput")
    LT1p_d = nc.dram_tensor("LT1p", [128, 1920], bf16, kind="ExternalInput")
    LU2p_d = nc.dram_tensor("LU2p", [128, 7680], bf16, kind="ExternalInput")
    LT2_d = nc.dram_tensor("LT2", [128, 12800], bf16, kind="ExternalInput")
    LL2_d = nc.dram_tensor("LL2", [128, 12800], bf16, kind="ExternalInput")
    gbs_d = [nc.dram_tensor(f"gb{i}", [1, 2 * F], f32, kind="ExternalInput")
             for i, F in enumerate([64, 32, 32])]
    b3r_d = nc.dram_tensor("b3r", [1, 480], f32, kind="ExternalInput")
    ydram = nc.dram_tensor("y", [128, 960], f32, kind="ExternalOutput")

    with tile.TileContext(nc) as tc:
        with (
            tc.tile_pool(name="wpool", bufs=1) as wpool,
            tc.tile_pool(name="actp", bufs=1) as actp,
            tc.tile_pool(name="misc", bufs=1) as miscp,
            tc.tile_pool(name="pslin", bufs=2, space="PSUM") as pslin,
            tc.tile_pool(name="psW", bufs=3, space="PSUM") as psW,
            tc.tile_pool(name="dram", bufs=1, space="DRAM") as dramp,
        ):
            # ================= SBUF tiles =================
            W1 = wpool.tile([128, 20480], bf16, tag="W1")      # fc2w
            W2 = wpool.tile([128, 8192], bf16, tag="W2")       # fc1w
            LU2p = wpool.tile([128, 7680], bf16, tag="LU2p2")
            LT2 = wpool.tile([128, 12800], bf16, tag="LT2")
            LL2 = wpool.tile([128, 12800], bf16, tag="LL2")
            wblk = wpool.tile([128, 1188], bf16, tag="wblk")
            LU0p = wpool.tile([128, 640], bf16, tag="LU0p")
            LT1p = wpool.tile([128, 1920], bf16, tag="LT1p")
            smalls = wpool.tile([128, 100], f32, tag="smalls")
            xT = wpool.tile([128, 16 * BL], bf16, tag="xT")
            gb_sb = [wpool.tile([1, 2 * F], f32, tag=f"gb{i}",
                                name=f"gb{i}")
                     for i, F in enumerate([64, 32, 32])]
            b3r = wpool.tile([1, 480], f32, tag="b3r")

            def WB(nm, w):
                return wblk[:, woffs[nm]:woffs[nm] + w]

            # ---- DMA issue order == consumption order (SP queue) ----
            nc.sync.dma_start(xT[:], xTp[:])
            nc.sync.dma_start(smalls[:], smalls_d[:])
            nc.sync.dma_start(W2[:, 0:8192], fc1w_d[:])
            for mc in range(4):
                nc.sync.dma_start(W1[:, mc * 5120:(mc + 1) * 5120], fc2w_d[mc][:])
            nc.sync.dma_start(wblk[:], wblk_d[:])
            nc.sync.dma_start(LU0p[:], LU0p_d[:])
            nc.sync.dma_start(LT1p[:], LT1p_d[:])
            for i in range(3):
                nc.sync.dma_start(gb_sb[i][:], gbs_d[i][:])
            nc.sync.dma_start(b3r[:], b3r_d[:])
            # Late-use loads are emitted at compute milestones via late_load()
            # (a dummy gate write gives each chunk DMA a data dependency, so
            # the transfers never sit in the DMA_ENGINES queue ahead of the
            # tiny BN-collective DMAs).

            def late_load(dst, dsrc, nchunks, csz, dep):
                for k in range(nchunks):
                    nc.vector.tensor_copy(dst[0:1, k * csz:k * csz + 1], dep)
                    nc.sync.dma_start(dst[:, k * csz:(k + 1) * csz],
                                      dsrc[:, k * csz:(k + 1) * csz])

            # ---- constants / act-table warm ----
            eps_t = miscp.tile([1, 1], f32, tag="eps")
            nc.gpsimd.memset(eps_t[:], EPS)
            ones8 = miscp.tile([8, 1], f32, tag="ones8")
            nc.gpsimd.memset(ones8[:], 1.0)
            onesr = miscp.tile([1, 128], bf16, tag="onesr")
            nc.gpsimd.memset(onesr[:], 1.0)
            b3rb = miscp.tile([1, 480], bf16, tag="b3rb")
            nc.vector.tensor_copy(b3rb[:], b3r[:])
            ident_f = miscp.tile([128, 128], f32, tag="identf")
            make_identity(nc, ident_f[:])
            warm = miscp.tile([1, 4], f32, tag="warm")
            nc.gpsimd.memset(warm[:], 1.0)
            nc.scalar.activation(warm[:, 0:1], warm[:, 0:1], AF.Relu)
            nc.scalar.activation(warm[:, 1:2], warm[:, 1:2], AF.Copy)
            nc.scalar.activation(warm[:, 2:3], warm[:, 2:3], AF.Sqrt)

            # round-robin copy engines for PSUM->SBUF evacuation
            cp_state = [0]

            def cp(dst, src):
                e = cp_state[0] % 2
                cp_state[0] += 1
                if e == 0:
                    nc.scalar.activation(dst, src, AF.Copy)
                else:
                    nc.vector.tensor_copy(dst, src)

            # ================= FC head (bf16) =================
            h1T = miscp.tile([128, 4 * BL], bf16, tag="h1T")
            ps1 = pslin.tile([128, 512], f32, tag="lin")
            for mt in range(4):
                for kt in range(16):
                    nc.tensor.matmul(
                        ps1[:, mt * BL:(mt + 1) * BL],
                        W2[:, kt * 512 + mt * 128: kt * 512 + (mt + 1) * 128],
                        xT[:, kt * BL:(kt + 1) * BL],
                        start=(kt == 0), stop=(kt == 15),
                        skip_group_check=True)
                nc.scalar.activation(
                    h1T[:, mt * BL:(mt + 1) * BL], ps1[:, mt * BL:(mt + 1) * BL],
                    AF.Relu, bias=smalls[:, 96 + mt:97 + mt])

            XF0 = actp.tile([128, 16 * 80], bf16, tag="XF0")
            for mc in range(4):
                ps2 = psW.tile([128, 1024], f32, tag="big")
                for mi in range(10):
                    for kt in range(4):
                        nc.tensor.matmul(
                            ps2[:, mi * BL:(mi + 1) * BL],
                            W1[:, mc * 5120 + kt * 1280 + mi * 128:
                               mc * 5120 + kt * 1280 + (mi + 1) * 128],
                            h1T[:, kt * BL:(kt + 1) * BL],
                            start=(kt == 0), stop=(kt == 3),
                            skip_group_check=True)
                # psum [(v0%2)*64+f, b] -> XF0[(b%2)*64+f, (b//2)*80 + v0]
                src4 = ps2[:, 0:320].rearrange("p (i g j) -> p i g j", g=16, j=2)
                dst4 = XF0[:].rearrange("p (g u q) -> p g u q", u=40, q=2)
                for p0 in range(2):
                    for j in range(2):
                        nc.scalar.activation(
                            dst4[j * 64:(j + 1) * 64, :,
                                 mc * 10:(mc + 1) * 10, p0]
                            .rearrange("p g i -> p i g"),
                            src4[p0 * 64:(p0 + 1) * 64, :, :, j],
                            AF.Copy)

            # ================= cheby layers c0-c2 (F-layout) =================
            XF_cur = XF0

            for li, cfg in enumerate(CFG):
                V, Vin, F = cfg.V, cfg.Vin, cfg.Fout
                BF, nS, nGp = cfg.BF, cfg.nS, cfg.nGp
                # L-matrix rhs tiles: [128, nS*V (L-part) | nS*V (2L^2-part)]
                if cfg.name == "c0":
                    Lr, lw = LU0p, 320
                elif cfg.name == "c1":
                    Lr, lw = LT1p, 320
                else:
                    Lr, lw = LU2p, 1280

                # ---- B/C linears into source-vertex space ----
                XB = actp.tile([128, 3072], bf16, tag="XB",
                               name="XB")[:, :nS * BF]
                XC = actp.tile([128, 3072], bf16, tag="XC",
                               name="XC")[:, :nS * BF]
                gpack = max(1, 512 // cfg.GF)
                for s in range(nS):
                    ssz = cfg.sps(s)
                    for dst, wnm in ((XB, f"B{li}"), (XC, f"C{li}")):
                        Wt = WB(wnm, cfg.GF)
                        for g0 in range(0, cfg.nG, gpack):
                            gn = min(gpack, cfg.nG - g0)
                            pc = pslin.tile([128, 512], f32, tag="lin")
                            for gi in range(gn):
                                g = g0 + gi
                                nc.tensor.matmul(
                                    pc[:ssz, gi * cfg.GF:(gi + 1) * cfg.GF],
                                    XF_cur[:, g * Vin + s * 128:
                                           g * Vin + s * 128 + ssz],
                                    Wt, start=True, stop=True,
                                    skip_group_check=True)
                            cp(dst[:ssz, s * BF + g0 * cfg.GF:
                                   s * BF + (g0 + gn) * cfg.GF],
                               pc[:ssz, :gn * cfg.GF])

                if li == 0:
                    # pace LU2p load: enqueued after c0's B/C copies, clear of
                    # the BN0 collective's DMA window
                    late_load(LU2p, LU2p_d, 4, 1920, XF0[0:1, 0:1])

                # ---- transposed spmm + copies + 512-wide stats windows ----
                XFn = actp.tile([128, nGp * V], bf16, tag=f"XFn{li}")
                FD = nGp * V
                nch = FD // 512
                bnst = miscp.tile([128, nch * 6], f32, tag=f"bnst{li}")
                stat_done = [0, 0]  # cols copied, windows emitted

                def emit_stats(done, XFn=XFn, bnst=bnst, sd=stat_done, nch=nch):
                    sd[0] = done
                    while sd[1] < nch and (sd[1] + 1) * 512 <= sd[0]:
                        ci = sd[1]
                        nc.vector.bn_stats(
                            bnst[:, ci * 6:(ci + 1) * 6],
                            XFn[:, ci * 512:(ci + 1) * 512])
                        sd[1] += 1

                def spmm_group(ps, pbase, gp, w0, wcw):
                    """Accumulate output block (gp, w0:w0+wcw) into ps cols
                    pbase:pbase+wcw (wcw <= 512)."""
                    for half, XS in ((0, XB), (1, XC)):
                        for s in range(nS):
                            ssz = cfg.sps(s)
                            nc.tensor.matmul(
                                ps[:, pbase:pbase + wcw],
                                XS[:ssz, s * BF + gp * 128:
                                   s * BF + (gp + 1) * 128],
                                Lr[:ssz, half * nS * lw + s * lw + w0:
                                   half * nS * lw + s * lw + w0 + wcw],
                                start=(half == 0 and s == 0), stop=False,
                                skip_group_check=True)
                    if cfg.name == "c1":
                        for dl in range(2):
                            nc.tensor.matmul(
                                ps[:, pbase:pbase + wcw],
                                WB(f"A1_{dl}", 128),
                                XF_cur[:, (2 * gp + dl) * Vin + w0:
                                       (2 * gp + dl) * Vin + w0 + wcw],
                                start=False, stop=(dl == 1),
                                skip_group_check=True)
                    else:
                        rhs = XF_cur[:, gp * Vin + w0 // 4:
                                     gp * Vin + w0 // 4 + wcw // 4]
                        rhs = rhs.broadcast_to([128, wcw // 4, 4])
                        nc.tensor.matmul(
                            ps[:, pbase:pbase + wcw], WB(f"A{li}", 128), rhs,
                            start=False, stop=True, skip_group_check=True)

                if V <= 512:
                    # pack 2 gp-blocks per PSUM tile (bank-aligned at 512)
                    for gpp in range(0, nGp, 2):
                        ps = psW.tile([128, 1024], f32, tag="big")
                        for gi in range(2):
                            spmm_group(ps, gi * 512, gpp + gi, 0, V)
                        cp(XFn[:, gpp * V:(gpp + 2) * V]
                           .rearrange("p (u w) -> p u w", w=V),
                           ps[:].rearrange("p (u w) -> p u w", w=512)[:, :, :V])
                        emit_stats((gpp + 2) * V)
                else:
                    # c2: V=1280 per gp -> tiles of 1024 + 256
                    for gp in range(nGp):
                        for w0 in (0, 1024):
                            wcw = min(1024, V - w0)
                            ps = psW.tile([128, 1024], f32, tag="big")
                            for nk in range(0, wcw, 512):
                                sub = min(512, wcw - nk)
                                spmm_group(ps, nk, gp, w0 + nk, sub)
                            cp(XFn[:, gp * V + w0: gp * V + w0 + wcw],
                               ps[:, :wcw])
                            emit_stats(gp * V + w0 + wcw)

                # ---- BN: partial sums -> AllGather -> scale/shift ----
                n_g = float(B * V)
                aggr = miscp.tile([128, 2], f32, tag="aggr")
                nc.vector.bn_aggr(
                    aggr[:], bnst[:].rearrange("p (c s) -> p c s", s=6))
                part = miscp.tile([128, 2], f32, tag="part")
                nc.vector.tensor_tensor(
                    out=part[:, 1:2], in0=aggr[:, 0:1], in1=aggr[:, 0:1],
                    op=ALU.mult)
                nc.vector.tensor_tensor(
                    out=part[:, 1:2], in0=part[:, 1:2], in1=aggr[:, 1:2],
                    op=ALU.add)
                fscl = float(FD) / n_g
                nc.vector.tensor_scalar_mul(part[:, 1:2], part[:, 1:2], fscl)
                nc.vector.tensor_scalar_mul(part[:, 0:1], aggr[:, 0:1], fscl)
                sel = smalls[:, 0:64] if F == 64 else smalls[:, 64:96]
                pst = pslin.tile([128, 512], f32, tag="lin")
                nc.tensor.matmul(pst[:1, :F], part[:, 0:1], sel,
                                 start=True, stop=True, skip_group_check=True)
                nc.tensor.matmul(pst[:1, F:2 * F], part[:, 1:2], sel,
                                 start=True, stop=True, skip_group_check=True)
                stats_l = miscp.tile([1, 2 * F], f32, tag="statl")
                nc.vector.tensor_copy(stats_l[:], pst[:1, :2 * F])
                bin_ = dramp.tile([1, 2 * F], f32, tag=f"arin{li}")
                bout = dramp.tile([NCORES, 2 * F], f32, tag=f"arout{li}")
                nc.gpsimd.dma_start(bin_[:], stats_l[:])
                nc.gpsimd.collective_compute(
                    "AllGather", ALU.bypass,
                    replica_groups=[list(range(NCORES))],
                    ins=[bin_.opt()], outs=[bout.opt()])
                statg8 = miscp.tile([NCORES, 2 * F], f32, tag="statg8")
                nc.gpsimd.dma_start(statg8[:], bout[:])
                psg = pslin.tile([128, 512], f32, tag="lin")
                nc.tensor.matmul(psg[:1, :2 * F], ones8[:], statg8[:],
                                 start=True, stop=True, skip_group_check=True)
                # s,t from global [mu | Ex^2] (psg)
                st = miscp.tile([1, 2 * F], f32, tag="st")
                tmp = miscp.tile([1, 2 * F], f32, tag="sttmp")
                mu2 = miscp.tile([1, F], f32, tag="mu2")
                nc.vector.tensor_copy(tmp[:, :2 * F], psg[:1, :2 * F])
                nc.vector.tensor_tensor(out=mu2[:], in0=tmp[:, 0:F],
                                        in1=tmp[:, 0:F], op=ALU.mult)
                nc.vector.tensor_tensor(out=tmp[:, F:2 * F],
                                        in0=tmp[:, F:2 * F],
                                        in1=mu2[:], op=ALU.subtract)
                nc.scalar.activation(tmp[:, F:2 * F], tmp[:, F:2 * F],
                                     AF.Sqrt, bias=eps_t[:])
                nc.vector.reciprocal(tmp[:, F:2 * F], tmp[:, F:2 * F])
                nc.vector.tensor_tensor(out=st[:, 0:F],
                                        in0=tmp[:, F:2 * F],
                                        in1=gb_sb[li][:, 0:F], op=ALU.mult)
                nc.vector.tensor_tensor(out=mu2[:], in0=tmp[:, 0:F],
                                        in1=st[:, 0:F], op=ALU.mult)
                nc.vector.tensor_tensor(out=st[:, F:2 * F],
                                        in0=gb_sb[li][:, F:2 * F],
                                        in1=mu2[:], op=ALU.subtract)
                pss = pslin.tile([128, 512], f32, tag="lin", name="pss")
                nc.tensor.transpose(pss[:2 * F, 0:1], st[:],
                                    ident_f[:1, :1])
                stc = miscp.tile([128, 2], f32, tag=f"stc{li}")
                for j in range(cfg.Gp):
                    nc.vector.tensor_copy(stc[j * F:(j + 1) * F, 0:1],
                                          pss[:F, 0:1])
                    nc.vector.tensor_copy(stc[j * F:(j + 1) * F, 1:2],
                                          pss[F:2 * F, 0:1])
                # chunked scale/shift + relu, split across Act and DVE
                def relu_chunk(ap, on_dve):
                    on_dve = False
                    if on_dve:
                        nc.vector.tensor_scalar(ap, ap, stc[:, 0:1],
                                                stc[:, 1:2], ALU.mult, ALU.add)
                        nc.vector.tensor_scalar_max(ap, ap, 0.0)
                    else:
                        nc.scalar.activation(ap, ap, AF.Relu,
                                             scale=stc[:, 0:1],
                                             bias=stc[:, 1:2])

                if li < 2:
                    # g-major contiguous chunks
                    csz = FD // 4
                    for rc in range(4):
                        relu_chunk(XFn[:, rc * csz:(rc + 1) * csz],
                                   on_dve=(rc % 2 == 1))
                else:
                    # s-major strided chunks so c3's (s)-pipeline can start
                    # after one chunk; DVE takes every third (strided = slow)
                    xv = XFn[:].rearrange("p (g w) -> p g w", w=V)
                    for sc in range(10):
                        relu_chunk(xv[:, :, sc * 128:(sc + 1) * 128],
                                   on_dve=(sc % 3 == 2))
                if li == 0:
                    late_load(LT2, LT2_d, 8, 1600, XFn[0:1, 0:1])
                elif li == 1:
                    late_load(LL2, LL2_d, 8, 1600, XFn[0:1, 0:1])
                XF_cur = XFn

            # ================= c3 (V-layout) + output =================
            # XF_cur = XF2 [128 (j4,c32), 8*1280], G=4, nG=8.
            # Per-s pipeline: relu chunk s -> B/C linears of source tile s ->
            # contributions of s accumulated into 2 persistent PSUM tiles
            # (t 0-4 and t 5-9), so the spmm overlaps the relu chunks.
            XB3 = actp.tile([128, 3072], bf16, tag="XB",
                            name="XB3")[:, :960]
            XC3 = actp.tile([128, 3072], bf16, tag="XC",
                            name="XC3")[:, :960]
            for s in range(10):
                pc = pslin.tile([128, 512], f32, tag="lin")
                for dst, wnm, off in ((XB3, "B3", 0), (XC3, "C3", 96)):
                    Wt = WB(wnm, 12)
                    for g in range(8):
                        nc.tensor.matmul(
                            pc[:, off + g * 12: off + (g + 1) * 12],
                            XF_cur[:, g * 1280 + s * 128:
                                   g * 1280 + (s + 1) * 128],
                            Wt, start=True, stop=True, skip_group_check=True)
                cp(XB3[:, s * 96:(s + 1) * 96], pc[:, 0:96])
                cp(XC3[:, s * 96:(s + 1) * 96], pc[:, 96:192])
            ysb = miscp.tile([128, 960], f32, tag="ysb")
            for t in range(10):
                pv = psW.tile([128, 1024], f32, tag="big")
                for s in range(10):
                    nc.tensor.matmul(
                        pv[:, 0:96],
                        LT2[:, s * 1280 + t * 128:s * 1280 + (t + 1) * 128],
                        XB3[:, s * 96:(s + 1) * 96],
                        start=(s == 0), stop=False, skip_group_check=True)
                for s in range(10):
                    nc.tensor.matmul(
                        pv[:, 0:96],
                        LL2[:, s * 1280 + t * 128:s * 1280 + (t + 1) * 128],
                        XC3[:, s * 96:(s + 1) * 96],
                        start=False, stop=False, skip_group_check=True)
                for g in range(8):
                    nc.tensor.matmul(
                        pv[:, g * 12:(g + 1) * 12],
                        XF_cur[:, g * 1280 + t * 128:g * 1280 + (t + 1) * 128],
                        WB("A3", 12),
                        start=False, stop=False, skip_group_check=True)
                nc.tensor.matmul(
                    pv[:, 0:96], onesr[:, :128], b3rb[:, 0:96],
                    start=False, stop=True, skip_group_check=True)
                cp(ysb[:, t * 96:(t + 1) * 96], pv[:, 0:96])
            nc.sync.dma_start(ydram[:, 0:480], ysb[:, 0:480])
            nc.sync.dma_start(ydram[:, 480:960], ysb[:, 480:960])

    nc.compile()
    return nc


def kernel(**inputs):
    import sys
    for p in ("/opt/trn_rl_repo", "/opt/trn_rl_repo/concourse"):
        if p not in sys.path:
            sys.path.insert(0, p)
    from concourse.bass_utils import run_bass_kernel_spmd
    import ml_dtypes

    host = _build_host(inputs)
    woffs = host.pop("_woffs")
    xT_full = host.pop("xTp_full")

    key = ("nc",)
    if key not in _CACHE:
        _CACHE[key] = _build_nc(woffs)
    nc = _CACHE[key]

    in_maps = []
    for c in range(NCORES):
        m = dict(host)
        xc = xT_full[:, c * BL:(c + 1) * BL]  # [2048, 32]
        m["xTp"] = np.ascontiguousarray(
            xc.reshape(16, 128, BL).transpose(1, 0, 2).reshape(128, 16 * BL)
        ).astype(ml_dtypes.bfloat16)
        in_maps.append(m)
    res = run_bass_kernel_spmd(nc, in_maps, core_ids=list(range(NCORES)))
    outs = []
    for c in range(NCORES):
        y = res.results[c]["y"].astype(np.float32)  # [128, 960]
        outs.append(y.reshape(128, 10, BL, 3).transpose(2, 1, 0, 3)
                    .reshape(BL, 1280, 3))
    return np.concatenate(outs, axis=0)


if __name__ == "__main__":
    import reference as R
    inp = R.setup_inputs()
    inp = {k: np.asarray(v) for k, v in inp.items()}
    act = kernel(**inp)
    exp = np.asarray(R.reference(**inp))
    err = np.linalg.norm(act - exp) / np.linalg.norm(exp)
    print("Relative error:", err)


# revision 18
# speedup vs baseline: 1.9153x; 1.0420x over previous
"""Trainium2 Bass kernel for nn_Graph_CNN_Feat_Mesh (Chebyshev GNN decoder).

Strategy (per-core, data-parallel over batch B=256 -> 32/core):
  - All spmms are dense matmuls on the tensor engine (PE) in bf16.
    For K=3 Chebyshev conv:  y = A(x) + L @ B(x) + (2 L^2) @ C(x)
    with A = W0-W2, B = W1, C = W2 applied per-vertex in feature space.
    For up4-preceded layers, replication is folded into the host-side
    matrices:  y = A(x_up) + (L U) @ B(x320) + (2 L^2 U) @ C(x320),
    so both spmms contract over the small pre-upsample vertex space.
  - Layers c0-c2 run the spmm TRANSPOSED (lhsT = feature tiles, rhs = L
    tiles), emitting the next layer's packed F-layout directly: no
    back-transposes.  The A-term accumulates into the same PSUM with a
    stride-0 broadcast rhs for the up4 replication.
  - BatchNorm (training mode, global batch stats) is exact: per-core
    partial sums are AllGather'd across the 8 cores (cheaper than
    AllReduce) and summed locally; scale/shift+relu is applied in column
    chunks feeding the next layer's matmuls incrementally.
  - FC head runs in bf16 with fp32 PSUM accumulation; weight DMAs are
    issued in consumption order and big late-use matrices (L2, 2*L2^2)
    alias the FC weight SBUF space (chunked so the tiny BN collective
    DMAs never queue behind a long transfer).
"""

import numpy as np

B = 256
NCORES = 8
BL = B // NCORES  # 32
EPS = 1e-5

_CACHE = {}


def _split_W(W):
    W = np.asarray(W, np.float32)
    return W[:, 0::3], W[:, 1::3], W[:, 2::3]


def _dense_L(rows, cols, vals, V):
    L = np.zeros((V, V), np.float32)
    np.add.at(L, (np.asarray(rows), np.asarray(cols)), np.asarray(vals, np.float32))
    return L


def _pad_rows(a, m):
    if a.shape[0] % m == 0:
        return a
    p = m - a.shape[0] % m
    return np.concatenate([a, np.zeros((p,) + a.shape[1:], a.dtype)], 0)


def _stiles(a):
    """[U, V] -> [128, nS*V] with s-tiles of 128 source rows side by side."""
    a = _pad_rows(np.ascontiguousarray(a), 128)
    nS = a.shape[0] // 128
    return np.concatenate([a[s * 128:(s + 1) * 128, :] for s in range(nS)], axis=1)


def _wbd(M, G, Fin, Fout):
    """Block-diagonal weight [128, G*Fout]; block j holds M.T ([Fin, Fout])."""
    out = np.zeros((128, G * Fout), np.float32)
    for j in range(G):
        out[j * Fin:(j + 1) * Fin, j * Fout:(j + 1) * Fout] = M.T
    return out


class _LCfg:
    def __init__(self, name, Vin, Vsp, V, Fin, Fout, up4):
        self.name = name
        self.Vin = Vin            # per-g input column span of XF
        self.Vsp = Vsp            # source vertex space of B/C linears
        self.V = V                # output vertex count
        self.Fin = Fin
        self.Fout = Fout
        self.G = 128 // Fin       # input batch packs
        self.nG = BL // self.G
        self.GF = self.G * Fout
        self.Gp = 128 // Fout     # output batch packs
        self.BF = BL * Fout
        self.nGp = self.BF // 128  # output 128-col blocks
        self.nS = (Vsp + 127) // 128
        self.up4 = up4

    def sps(self, s):
        return min(128, self.Vsp - s * 128)


CFG = [
    _LCfg("c0", 80, 80, 320, 64, 64, True),
    _LCfg("c1", 320, 320, 320, 64, 32, False),
    _LCfg("c2", 320, 320, 1280, 32, 32, True),
]
# c3 (V-layout output layer): Fin=32, Fout=3, G=4, V=Vsp=1280


def _build_host(inputs):
    import ml_dtypes
    bf = ml_dtypes.bfloat16
    f32 = np.float32
    d = {}

    # ---- FC head ----
    xT = np.ascontiguousarray(np.asarray(inputs["x"], f32).T)  # [2048, 256]
    d["xTp_full"] = xT  # sliced + packed per core in kernel()
    fc1wT = np.ascontiguousarray(np.asarray(inputs["fc1_w"], f32).T)  # [2048, 512]
    d["fc1w"] = np.ascontiguousarray(
        fc1wT.reshape(16, 128, 512).transpose(1, 0, 2).reshape(128, 16 * 512)
    ).astype(bf)
    fc2wT = np.ascontiguousarray(np.asarray(inputs["fc2_w"], f32).T)  # [512, 5120]
    f2 = fc2wT.reshape(4, 128, 5120)
    for mc in range(4):
        d[f"fc2w{mc}"] = np.ascontiguousarray(
            f2[:, :, mc * 1280:(mc + 1) * 1280].transpose(1, 0, 2).reshape(128, 4 * 1280)
        ).astype(bf)
    smalls = np.zeros((128, 100), f32)
    for j in range(2):
        smalls[j * 64:(j + 1) * 64, 0:64] += np.eye(64, dtype=f32)
    for j in range(4):
        smalls[j * 32:(j + 1) * 32, 64:96] += np.eye(32, dtype=f32)
    smalls[:, 96:100] = np.asarray(inputs["fc1_b"], f32).reshape(4, 128).T
    d["smalls"] = smalls

    # ---- L matrices ----
    L1 = _dense_L(inputs["L1_rows"], inputs["L1_cols"], inputs["L1_vals"], 320)
    L2 = _dense_L(inputs["L2_rows"], inputs["L2_cols"], inputs["L2_vals"], 1280)
    U1 = np.repeat(np.eye(80, dtype=f32), 4, axis=0)    # [320, 80]
    U2 = np.repeat(np.eye(320, dtype=f32), 4, axis=0)   # [1280, 320]
    LU0 = (L1 @ U1).T                                   # [80, 320]
    LLU0 = 2.0 * (L1 @ (L1 @ U1)).T
    d["LU0p"] = _pad_rows(np.concatenate([LU0, LLU0], axis=1), 128).astype(bf)
    LT1 = _stiles(L1.T)                                 # [128, 3*320]
    LL1 = _stiles(2.0 * (L1 @ L1).T)
    d["LT1p"] = np.concatenate([LT1, LL1], axis=1).astype(bf)
    LU2 = _stiles((L2 @ U2).T)                          # [128, 3*1280]
    LLU2 = _stiles(2.0 * (L2 @ (L2 @ U2)).T)
    d["LU2p"] = np.concatenate([LU2, LLU2], axis=1).astype(bf)
    d["LT2"] = _stiles(L2.T).astype(bf)                 # [128, 10*1280]
    d["LL2"] = _stiles(2.0 * (L2 @ L2).T).astype(bf)

    # ---- Chebyshev linear weight blocks ----
    blks = []
    offs = {}

    def add(nm, arr):
        offs[nm] = sum(b.shape[1] for b in blks)
        blks.append(arr)

    for li, (cfg, wn) in enumerate(zip(CFG, ["cl0_w", "cl1_w", "cl2_w"])):
        W0, W1, W2 = _split_W(inputs[wn])
        A = W0 - W2
        add(f"B{li}", _wbd(W1, cfg.G, cfg.Fin, cfg.Fout))
        add(f"C{li}", _wbd(W2, cfg.G, cfg.Fin, cfg.Fout))
        if cfg.name == "c1":
            for dl in range(2):
                M = np.zeros((128, 128), f32)
                for j in range(2):
                    M[j * 64:(j + 1) * 64,
                      (2 * dl + j) * 32:(2 * dl + j + 1) * 32] = A.T
                add(f"A1_{dl}", M)
        else:
            add(f"A{li}", _wbd(A, cfg.G, cfg.Fin, cfg.Fout))
    W0, W1, W2 = _split_W(inputs["cl3_w"])
    add("B3", _wbd(W1, 4, 32, 3))
    add("C3", _wbd(W2, 4, 32, 3))
    add("A3", _wbd(W0 - W2, 4, 32, 3))
    d["wblk"] = np.concatenate(blks, axis=1).astype(bf)
    d["_woffs"] = offs  # not uploaded

    for i, (g, b) in enumerate([("bn0_g", "bn0_b"), ("bn1_g", "bn1_b"),
                                ("bn2_g", "bn2_b")]):
        gb = np.concatenate([np.asarray(inputs[g], f32),
                             np.asarray(inputs[b], f32)])
        d[f"gb{i}"] = np.ascontiguousarray(gb[None, :])  # [1, 2F]
    b3 = np.asarray(inputs["cl3_b"], f32)
    d["b3r"] = np.ascontiguousarray(np.tile(b3, 160)[None, :])  # [1, 480]
    return d


def _build_nc(woffs):
    import sys
    for p in ("/opt/trn_rl_repo", "/opt/trn_rl_repo/concourse"):
        if p not in sys.path:
            sys.path.insert(0, p)
    import concourse.bass as bass  # noqa
    import concourse.mybir as mybir
    import concourse.tile as tile
    from concourse import bacc
    from concourse.masks import make_identity

    f32 = mybir.dt.float32
    bf16 = mybir.dt.bfloat16
    AF = mybir.ActivationFunctionType
    ALU = mybir.AluOpType

    nc = bacc.Bacc(None, target_bir_lowering=False)

    xTp = nc.dram_tensor("xTp", [128, 16 * BL], bf16, kind="ExternalInput")
    smalls_d = nc.dram_tensor("smalls", [128, 100], f32, kind="ExternalInput")
    fc1w_d = nc.dram_tensor("fc1w", [128, 16 * 512], bf16, kind="ExternalInput")
    fc2w_d = [nc.dram_tensor(f"fc2w{mc}", [128, 4 * 1280], bf16,
                             kind="ExternalInput") for mc in range(4)]
    wblk_d = nc.dram_tensor("wblk", [128, 1188], bf16, kind="ExternalInput")
    LU0p_d = nc.dram_tensor("LU0p", [128, 640], bf16, kind="ExternalInput")
    LT1p_d = nc.dram_tensor("LT1p", [128, 1920], bf16, kind="ExternalInput")
    LU2p_d = nc.dram_tensor("LU2p", [128, 7680], bf16, kind="ExternalInput")
    LT2_d = nc.dram_tensor("LT2", [128, 12800], bf16, kind="ExternalInput")
    LL2_d = nc.dram_tensor("LL2", [128, 12800], bf16, kind="ExternalInput")
    gbs_d = [nc.dram_tensor(f"gb{i}", [1, 2 * F], f32, kind="ExternalInput")
             for i, F in enumerate([64, 32, 32])]
    b3r_d = nc.dram_tensor("b3r", [1, 480], f32, kind="ExternalInput")
    ydram = nc.dram_tensor("y", [128, 960], f32, kind="ExternalOutput")

    with tile.TileContext(nc) as tc:
        with (
            tc.tile_pool(name="wpool", bufs=1) as wpool,
            tc.tile_pool(name="actp", bufs=1) as actp,
            tc.tile_pool(name="misc", bufs=1) as miscp,
            tc.tile_pool(name="pslin", bufs=2, space="PSUM") as pslin,
            tc.tile_pool(name="psW", bufs=3, space="PSUM") as psW,
            tc.tile_pool(name="dram", bufs=1, space="DRAM") as dramp,
        ):
            # ================= SBUF tiles =================
            W1 = wpool.tile([128, 20480], bf16, tag="W1")      # fc2w
            W2 = wpool.tile([128, 8192], bf16, tag="W2")       # fc1w
            LU2p = wpool.tile([128, 7680], bf16, tag="LU2p2")
            LT2 = wpool.tile([128, 12800], bf16, tag="LT2")
            LL2 = wpool.tile([128, 12800], bf16, tag="LL2")
            wblk = wpool.tile([128, 1188], bf16, tag="wblk")
            LU0p = wpool.tile([128, 640], bf16, tag="LU0p")
            LT1p = wpool.tile([128, 1920], bf16, tag="LT1p")
            smalls = wpool.tile([128, 100], f32, tag="smalls")
            xT = wpool.tile([128, 16 * BL], bf16, tag="xT")
            gb_sb = [wpool.tile([1, 2 * F], f32, tag=f"gb{i}",
                                name=f"gb{i}")
                     for i, F in enumerate([64, 32, 32])]
            b3r = wpool.tile([1, 480], f32, tag="b3r")

            def WB(nm, w):
                return wblk[:, woffs[nm]:woffs[nm] + w]

            # ---- DMA issue order == consumption order (SP queue) ----
            nc.sync.dma_start(xT[:], xTp[:])
            nc.sync.dma_start(smalls[:], smalls_d[:])
            nc.sync.dma_start(W2[:, 0:8192], fc1w_d[:])
            for mc in range(4):
                nc.sync.dma_start(W1[:, mc * 5120:(mc + 1) * 5120], fc2w_d[mc][:])
            nc.sync.dma_start(wblk[:], wblk_d[:])
            nc.sync.dma_start(LU0p[:], LU0p_d[:])
            nc.sync.dma_start(LT1p[:], LT1p_d[:])
            for i in range(3):
                nc.sync.dma_start(gb_sb[i][:], gbs_d[i][:])
            nc.sync.dma_start(b3r[:], b3r_d[:])
            # Late-use loads are emitted at compute milestones via late_load()
            # (a dummy gate write gives each chunk DMA a data dependency, so
            # the transfers never sit in the DMA_ENGINES queue ahead of the
            # tiny BN-collective DMAs).

            def late_load(dst, dsrc, nchunks, csz, dep):
                for k in range(nchunks):
                    nc.vector.tensor_copy(dst[0:1, k * csz:k * csz + 1], dep)
                    nc.sync.dma_start(dst[:, k * csz:(k + 1) * csz],
                                      dsrc[:, k * csz:(k + 1) * csz])

            # ---- constants / act-table warm ----
            eps_t = miscp.tile([1, 1], f32, tag="eps")
            nc.gpsimd.memset(eps_t[:], EPS)
            ones8 = miscp.tile([8, 1], f32, tag="ones8")
            nc.gpsimd.memset(ones8[:], 1.0)
            onesr = miscp.tile([1, 128], bf16, tag="onesr")
            nc.gpsimd.memset(onesr[:], 1.0)
            b3rb = miscp.tile([1, 480], bf16, tag="b3rb")
            nc.vector.tensor_copy(b3rb[:], b3r[:])
            ident_f = miscp.tile([128, 128], f32, tag="identf")
            make_identity(nc, ident_f[:])
            warm = miscp.tile([1, 4], f32, tag="warm")
            nc.gpsimd.memset(warm[:], 1.0)
            nc.scalar.activation(warm[:, 0:1], warm[:, 0:1], AF.Relu)
            nc.scalar.activation(warm[:, 1:2], warm[:, 1:2], AF.Copy)
            nc.scalar.activation(warm[:, 2:3], warm[:, 2:3], AF.Sqrt)

            # round-robin copy engines for PSUM->SBUF evacuation
            cp_state = [0]

            def cp(dst, src):
                e = cp_state[0] % 2
                cp_state[0] += 1
                if e == 0:
                    nc.scalar.activation(dst, src, AF.Copy)
                else:
                    nc.vector.tensor_copy(dst, src)

            # ================= FC head (bf16) =================
            h1T = miscp.tile([128, 4 * BL], bf16, tag="h1T")
            ps1 = pslin.tile([128, 512], f32, tag="lin")
            for mt in range(4):
                for kt in range(16):
                    nc.tensor.matmul(
                        ps1[:, mt * BL:(mt + 1) * BL],
                        W2[:, kt * 512 + mt * 128: kt * 512 + (mt + 1) * 128],
                        xT[:, kt * BL:(kt + 1) * BL],
                        start=(kt == 0), stop=(kt == 15),
                        skip_group_check=True)
                nc.scalar.activation(
                    h1T[:, mt * BL:(mt + 1) * BL], ps1[:, mt * BL:(mt + 1) * BL],
                    AF.Relu, bias=smalls[:, 96 + mt:97 + mt])

            XF0 = actp.tile([128, 16 * 80], bf16, tag="XF0")
            for mc in range(4):
                ps2 = psW.tile([128, 1024], f32, tag="big")
                for mi in range(10):
                    for kt in range(4):
                        nc.tensor.matmul(
                            ps2[:, mi * BL:(mi + 1) * BL],
                            W1[:, mc * 5120 + kt * 1280 + mi * 128:
                               mc * 5120 + kt * 1280 + (mi + 1) * 128],
                            h1T[:, kt * BL:(kt + 1) * BL],
                            start=(kt == 0), stop=(kt == 3),
                            skip_group_check=True)
                # psum [(v0%2)*64+f, b] -> XF0[(b%2)*64+f, (b//2)*80 + v0]
                src4 = ps2[:, 0:320].rearrange("p (i g j) -> p i g j", g=16, j=2)
                dst4 = XF0[:].rearrange("p (g u q) -> p g u q", u=40, q=2)
                for p0 in range(2):
                    for j in range(2):
                        nc.scalar.activation(
                            dst4[j * 64:(j + 1) * 64, :,
                                 mc * 10:(mc + 1) * 10, p0]
                            .rearrange("p g i -> p i g"),
                            src4[p0 * 64:(p0 + 1) * 64, :, :, j],
                            AF.Copy)

            # ================= cheby layers c0-c2 (F-layout) =================
            XF_cur = XF0

            for li, cfg in enumerate(CFG):
                V, Vin, F = cfg.V, cfg.Vin, cfg.Fout
                BF, nS, nGp = cfg.BF, cfg.nS, cfg.nGp
                # L-matrix rhs tiles: [128, nS*V (L-part) | nS*V (2L^2-part)]
                if cfg.name == "c0":
                    Lr, lw = LU0p, 320
                elif cfg.name == "c1":
                    Lr, lw = LT1p, 320
                else:
                    Lr, lw = LU2p, 1280

                # ---- B/C linears into source-vertex space ----
                XB = actp.tile([128, 3072], bf16, tag="XB",
                               name="XB")[:, :nS * BF]
                XC = actp.tile([128, 3072], bf16, tag="XC",
                               name="XC")[:, :nS * BF]
                gpack = max(1, 512 // cfg.GF)
                for s in range(nS):
                    ssz = cfg.sps(s)
                    for dst, wnm in ((XB, f"B{li}"), (XC, f"C{li}")):
                        Wt = WB(wnm, cfg.GF)
                        for g0 in range(0, cfg.nG, gpack):
                            gn = min(gpack, cfg.nG - g0)
                            pc = pslin.tile([128, 512], f32, tag="lin")
                            for gi in range(gn):
                                g = g0 + gi
                                nc.tensor.matmul(
                                    pc[:ssz, gi * cfg.GF:(gi + 1) * cfg.GF],
                                    XF_cur[:, g * Vin + s * 128:
                                           g * Vin + s * 128 + ssz],
                                    Wt, start=True, stop=True,
                                    skip_group_check=True)
                            cp(dst[:ssz, s * BF + g0 * cfg.GF:
                                   s * BF + (g0 + gn) * cfg.GF],
                               pc[:ssz, :gn * cfg.GF])

                if li == 0:
                    # pace LU2p load: enqueued after c0's B/C copies, clear of
                    # the BN0 collective's DMA window
                    late_load(LU2p, LU2p_d, 4, 1920, XF0[0:1, 0:1])

                # ---- transposed spmm + copies + 512-wide stats windows ----
                XFn = actp.tile([128, nGp * V], bf16, tag=f"XFn{li}")
                FD = nGp * V
                nch = FD // 512
                bnst = miscp.tile([128, nch * 6], f32, tag=f"bnst{li}")
                stat_done = [0, 0]  # cols copied, windows emitted

                def emit_stats(done, XFn=XFn, bnst=bnst, sd=stat_done, nch=nch):
                    sd[0] = done
                    while sd[1] < nch and (sd[1] + 1) * 512 <= sd[0]:
                        ci = sd[1]
                        nc.vector.bn_stats(
                            bnst[:, ci * 6:(ci + 1) * 6],
                            XFn[:, ci * 512:(ci + 1) * 512])
                        sd[1] += 1

                def spmm_group(ps, pbase, gp, w0, wcw):
                    """Accumulate output block (gp, w0:w0+wcw) into ps cols
                    pbase:pbase+wcw (wcw <= 512)."""
                    for half, XS in ((0, XB), (1, XC)):
                        for s in range(nS):
                            ssz = cfg.sps(s)
                            nc.tensor.matmul(
                                ps[:, pbase:pbase + wcw],
                                XS[:ssz, s * BF + gp * 128:
                                   s * BF + (gp + 1) * 128],
                                Lr[:ssz, half * nS * lw + s * lw + w0:
                                   half * nS * lw + s * lw + w0 + wcw],
                                start=(half == 0 and s == 0), stop=False,
                                skip_group_check=True)
                    if cfg.name == "c1":
                        for dl in range(2):
                            nc.tensor.matmul(
                                ps[:, pbase:pbase + wcw],
                                WB(f"A1_{dl}", 128),
                                XF_cur[:, (2 * gp + dl) * Vin + w0:
                                       (2 * gp + dl) * Vin + w0 + wcw],
                                start=False, stop=(dl == 1),
                                skip_group_check=True)
                    else:
                        rhs = XF_cur[:, gp * Vin + w0 // 4:
                                     gp * Vin + w0 // 4 + wcw // 4]
                        rhs = rhs.broadcast_to([128, wcw // 4, 4])
                        nc.tensor.matmul(
                            ps[:, pbase:pbase + wcw], WB(f"A{li}", 128), rhs,
                            start=False, stop=True, skip_group_check=True)

                if V <= 512:
                    # pack 2 gp-blocks per PSUM tile (bank-aligned at 512)
                    for gpp in range(0, nGp, 2):
                        ps = psW.tile([128, 1024], f32, tag="big")
                        for gi in range(2):
                            spmm_group(ps, gi * 512, gpp + gi, 0, V)
                        cp(XFn[:, gpp * V:(gpp + 2) * V]
                           .rearrange("p (u w) -> p u w", w=V),
                           ps[:].rearrange("p (u w) -> p u w", w=512)[:, :, :V])
                        emit_stats((gpp + 2) * V)
                else:
                    # c2: V=1280 per gp -> tiles of 1024 + 256
                    for gp in range(nGp):
                        for w0 in (0, 1024):
                            wcw = min(1024, V - w0)
                            ps = psW.tile([128, 1024], f32, tag="big")
                            for nk in range(0, wcw, 512):
                                sub = min(512, wcw - nk)
                                spmm_group(ps, nk, gp, w0 + nk, sub)
                            cp(XFn[:, gp * V + w0: gp * V + w0 + wcw],
                               ps[:, :wcw])
                            emit_stats(gp * V + w0 + wcw)

                # ---- BN: partial sums -> AllGather -> scale/shift ----
                n_g = float(B * V)
                aggr = miscp.tile([128, 2], f32, tag="aggr")
                nc.vector.bn_aggr(
                    aggr[:], bnst[:].rearrange("p (c s) -> p c s", s=6))
                part = miscp.tile([128, 2], f32, tag="part")
                nc.vector.tensor_tensor(
                    out=part[:, 1:2], in0=aggr[:, 0:1], in1=aggr[:, 0:1],
                    op=ALU.mult)
                nc.vector.tensor_tensor(
                    out=part[:, 1:2], in0=part[:, 1:2], in1=aggr[:, 1:2],
                    op=ALU.add)
                fscl = float(FD) / n_g
                nc.vector.tensor_scalar_mul(part[:, 1:2], part[:, 1:2], fscl)
                nc.vector.tensor_scalar_mul(part[:, 0:1], aggr[:, 0:1], fscl)
                sel = smalls[:, 0:64] if F == 64 else smalls[:, 64:96]
                pst = pslin.tile([128, 512], f32, tag="lin")
                nc.tensor.matmul(pst[:1, :F], part[:, 0:1], sel,
                                 start=True, stop=True, skip_group_check=True)
                nc.tensor.matmul(pst[:1, F:2 * F], part[:, 1:2], sel,
                                 start=True, stop=True, skip_group_check=True)
                stats_l = miscp.tile([1, 2 * F], f32, tag="statl")
                nc.vector.tensor_copy(stats_l[:], pst[:1, :2 * F])
                bin_ = dramp.tile([1, 2 * F], f32, tag=f"arin{li}")
                bout = dramp.tile([NCORES, 2 * F], f32, tag=f"arout{li}")
                nc.gpsimd.dma_start(bin_[:], stats_l[:])
                nc.gpsimd.collective_compute(
                    "AllGather", ALU.bypass,
                    replica_groups=[list(range(NCORES))],
                    ins=[bin_.opt()], outs=[bout.opt()])
                statg8 = miscp.tile([NCORES, 2 * F], f32, tag="statg8")
                nc.gpsimd.dma_start(statg8[:], bout[:])
                psg = pslin.tile([128, 512], f32, tag="lin")
                nc.tensor.matmul(psg[:1, :2 * F], ones8[:], statg8[:],
                                 start=True, stop=True, skip_group_check=True)
                # s,t from global [mu | Ex^2] (psg)
                st = miscp.tile([1, 2 * F], f32, tag="st")
                tmp = miscp.tile([1, 2 * F], f32, tag="sttmp")
                mu2 = miscp.tile([1, F], f32, tag="mu2")
                nc.vector.tensor_copy(tmp[:, :2 * F], psg[:1, :2 * F])
                nc.vector.tensor_tensor(out=mu2[:], in0=tmp[:, 0:F],
                                        in1=tmp[:, 0:F], op=ALU.mult)
                nc.vector.tensor_tensor(out=tmp[:, F:2 * F],
                                        in0=tmp[:, F:2 * F],
                                        in1=mu2[:], op=ALU.subtract)
                nc.scalar.activation(tmp[:, F:2 * F], tmp[:, F:2 * F],
                                     AF.Sqrt, bias=eps_t[:])
                nc.vector.reciprocal(tmp[:, F:2 * F], tmp[:, F:2 * F])
                nc.vector.tensor_tensor(out=st[:, 0:F],
                                        in0=tmp[:, F:2 * F],
                                        in1=gb_sb[li][:, 0:F], op=ALU.mult)
                nc.vector.tensor_tensor(out=mu2[:], in0=tmp[:, 0:F],
                                        in1=st[:, 0:F], op=ALU.mult)
                nc.vector.tensor_tensor(out=st[:, F:2 * F],
                                        in0=gb_sb[li][:, F:2 * F],
                                        in1=mu2[:], op=ALU.subtract)
                pss = pslin.tile([128, 512], f32, tag="lin", name="pss")
                nc.tensor.transpose(pss[:2 * F, 0:1], st[:],
                                    ident_f[:1, :1])
                stc = miscp.tile([128, 2], f32, tag=f"stc{li}")
                for j in range(cfg.Gp):
                    nc.vector.tensor_copy(stc[j * F:(j + 1) * F, 0:1],
                                          pss[:F, 0:1])
                    nc.vector.tensor_copy(stc[j * F:(j + 1) * F, 1:2],
                                          pss[F:2 * F, 0:1])
                # chunked scale/shift + relu, split across Act/DVE/Pool
                # (contiguous g-aligned chunks: subtile deps track them)
                def relu_chunk(ap, eng):
                    if eng == 1:
                        nc.vector.tensor_scalar(ap, ap, stc[:, 0:1],
                                                stc[:, 1:2], ALU.mult, ALU.add)
                        nc.vector.tensor_scalar_max(ap, ap, 0.0)
                    elif eng == 2:
                        nc.gpsimd.tensor_scalar(ap, ap, stc[:, 0:1],
                                                stc[:, 1:2], ALU.mult, ALU.add)
                        nc.gpsimd.tensor_scalar_max(ap, ap, 0.0)
                    else:
                        nc.scalar.activation(ap, ap, AF.Relu,
                                             scale=stc[:, 0:1],
                                             bias=stc[:, 1:2])

                csz = FD // 8
                # engine shares ~ inverse throughput: Act 4, DVE 3, Pool 1
                sched = [0, 1, 0, 1, 2, 0, 1, 0]
                for rc in range(8):
                    relu_chunk(XFn[:, rc * csz:(rc + 1) * csz], sched[rc])
                if li == 0:
                    late_load(LT2, LT2_d, 8, 1600, XFn[0:1, 0:1])
                elif li == 1:
                    late_load(LL2, LL2_d, 8, 1600, XFn[0:1, 0:1])
                XF_cur = XFn

            # ================= c3 (V-layout) + output =================
            # XF_cur = XF2 [128 (j4,c32), 8*1280], G=4, nG=8.
            # Per-s pipeline: relu chunk s -> B/C linears of source tile s ->
            # contributions of s accumulated into 2 persistent PSUM tiles
            # (t 0-4 and t 5-9), so the spmm overlaps the relu chunks.
            XB3 = actp.tile([128, 3072], bf16, tag="XB",
                            name="XB3")[:, :960]
            XC3 = actp.tile([128, 3072], bf16, tag="XC",
                            name="XC3")[:, :960]
            for s in range(10):
                pc = pslin.tile([128, 512], f32, tag="lin")
                for dst, wnm, off in ((XB3, "B3", 0), (XC3, "C3", 96)):
                    Wt = WB(wnm, 12)
                    for g in range(8):
                        nc.tensor.matmul(
                            pc[:, off + g * 12: off + (g + 1) * 12],
                            XF_cur[:, g * 1280 + s * 128:
                                   g * 1280 + (s + 1) * 128],
                            Wt, start=True, stop=True, skip_group_check=True)
                cp(XB3[:, s * 96:(s + 1) * 96], pc[:, 0:96])
                cp(XC3[:, s * 96:(s + 1) * 96], pc[:, 96:192])
            ysb = miscp.tile([128, 960], f32, tag="ysb")
            for t in range(10):
                pv = psW.tile([128, 1024], f32, tag="big")
                for s in range(10):
                    nc.tensor.matmul(
                        pv[:, 0:96],
                        LT2[:, s * 1280 + t * 128:s * 1280 + (t + 1) * 128],
                        XB3[:, s * 96:(s + 1) * 96],
                        start=(s == 0), stop=False, skip_group_check=True)
                for s in range(10):
                    nc.tensor.matmul(
                        pv[:, 0:96],
                        LL2[:, s * 1280 + t * 128:s * 1280 + (t + 1) * 128],
                        XC3[:, s * 96:(s + 1) * 96],
                        start=False, stop=False, skip_group_check=True)
                for g in range(8):
                    nc.tensor.matmul(
                        pv[:, g * 12:(g + 1) * 12],
                        XF_cur[:, g * 1280 + t * 128:g * 1280 + (t + 1) * 128],
                        WB("A3", 12),
                        start=False, stop=False, skip_group_check=True)
                nc.tensor.matmul(
                    pv[:, 0:96], onesr[:, :128], b3rb[:, 0:96],
                    start=False, stop=True, skip_group_check=True)
                cp(ysb[:, t * 96:(t + 1) * 96], pv[:, 0:96])
            nc.sync.dma_start(ydram[:, 0:480], ysb[:, 0:480])
            nc.sync.dma_start(ydram[:, 480:960], ysb[:, 480:960])

    nc.compile()
    return nc


def kernel(**inputs):
    import sys
    for p in ("/opt/trn_rl_repo", "/opt/trn_rl_repo/concourse"):
        if p not in sys.path:
            sys.path.insert(0, p)
    from concourse.bass_utils import run_bass_kernel_spmd
    import ml_dtypes

    host = _build_host(inputs)
    woffs = host.pop("_woffs")
    xT_full = host.pop("xTp_full")

    key = ("nc",)
    if key not in _CACHE:
        _CACHE[key] = _build_nc(woffs)
    nc = _CACHE[key]

    in_maps = []
    for c in range(NCORES):
        m = dict(host)
        xc = xT_full[:, c * BL:(c + 1) * BL]  # [2048, 32]
        m["xTp"] = np.ascontiguousarray(
            xc.reshape(16, 128, BL).transpose(1, 0, 2).reshape(128, 16 * BL)
        ).astype(ml_dtypes.bfloat16)
        in_maps.append(m)
    res = run_bass_kernel_spmd(nc, in_maps, core_ids=list(range(NCORES)))
    outs = []
    for c in range(NCORES):
        y = res.results[c]["y"].astype(np.float32)  # [128, 960]
        outs.append(y.reshape(128, 10, BL, 3).transpose(2, 1, 0, 3)
                    .reshape(BL, 1280, 3))
    return np.concatenate(outs, axis=0)


if __name__ == "__main__":
    import reference as R
    inp = R.setup_inputs()
    inp = {k: np.asarray(v) for k, v in inp.items()}
    act = kernel(**inp)
    exp = np.asarray(R.reference(**inp))
    err = np.linalg.norm(act - exp) / np.linalg.norm(exp)
    print("Relative error:", err)


# revision 20
# speedup vs baseline: 1.9477x; 1.0169x over previous
"""Trainium2 Bass kernel for nn_Graph_CNN_Feat_Mesh (Chebyshev GNN decoder).

Strategy (per-core, data-parallel over batch B=256 -> 32/core):
  - All spmms are dense matmuls on the tensor engine (PE) in bf16.
    For K=3 Chebyshev conv:  y = A(x) + L @ B(x) + (2 L^2) @ C(x)
    with A = W0-W2, B = W1, C = W2 applied per-vertex in feature space.
    For up4-preceded layers, replication is folded into the host-side
    matrices:  y = A(x_up) + (L U) @ B(x320) + (2 L^2 U) @ C(x320),
    so both spmms contract over the small pre-upsample vertex space.
  - Layers c0-c2 run the spmm TRANSPOSED (lhsT = feature tiles, rhs = L
    tiles), emitting the next layer's packed F-layout directly: no
    back-transposes.  The A-term accumulates into the same PSUM with a
    stride-0 broadcast rhs for the up4 replication.
  - BatchNorm (training mode, global batch stats) is exact: per-core
    partial sums are AllGather'd across the 8 cores (cheaper than
    AllReduce) and summed locally; scale/shift+relu is applied in column
    chunks feeding the next layer's matmuls incrementally.
  - FC head runs in bf16 with fp32 PSUM accumulation; weight DMAs are
    issued in consumption order and big late-use matrices (L2, 2*L2^2)
    alias the FC weight SBUF space (chunked so the tiny BN collective
    DMAs never queue behind a long transfer).
"""

import numpy as np

B = 256
NCORES = 8
BL = B // NCORES  # 32
EPS = 1e-5

_CACHE = {}


def _split_W(W):
    W = np.asarray(W, np.float32)
    return W[:, 0::3], W[:, 1::3], W[:, 2::3]


def _dense_L(rows, cols, vals, V):
    L = np.zeros((V, V), np.float32)
    np.add.at(L, (np.asarray(rows), np.asarray(cols)), np.asarray(vals, np.float32))
    return L


def _pad_rows(a, m):
    if a.shape[0] % m == 0:
        return a
    p = m - a.shape[0] % m
    return np.concatenate([a, np.zeros((p,) + a.shape[1:], a.dtype)], 0)


def _stiles(a):
    """[U, V] -> [128, nS*V] with s-tiles of 128 source rows side by side."""
    a = _pad_rows(np.ascontiguousarray(a), 128)
    nS = a.shape[0] // 128
    return np.concatenate([a[s * 128:(s + 1) * 128, :] for s in range(nS)], axis=1)


def _wbd(M, G, Fin, Fout):
    """Block-diagonal weight [128, G*Fout]; block j holds M.T ([Fin, Fout])."""
    out = np.zeros((128, G * Fout), np.float32)
    for j in range(G):
        out[j * Fin:(j + 1) * Fin, j * Fout:(j + 1) * Fout] = M.T
    return out


class _LCfg:
    def __init__(self, name, Vin, Vsp, V, Fin, Fout, up4):
        self.name = name
        self.Vin = Vin            # per-g input column span of XF
        self.Vsp = Vsp            # source vertex space of B/C linears
        self.V = V                # output vertex count
        self.Fin = Fin
        self.Fout = Fout
        self.G = 128 // Fin       # input batch packs
        self.nG = BL // self.G
        self.GF = self.G * Fout
        self.Gp = 128 // Fout     # output batch packs
        self.BF = BL * Fout
        self.nGp = self.BF // 128  # output 128-col blocks
        self.nS = (Vsp + 127) // 128
        self.up4 = up4

    def sps(self, s):
        return min(128, self.Vsp - s * 128)


CFG = [
    _LCfg("c0", 80, 80, 320, 64, 64, True),
    _LCfg("c1", 320, 320, 320, 64, 32, False),
    _LCfg("c2", 320, 320, 1280, 32, 32, True),
]
# c3 (V-layout output layer): Fin=32, Fout=3, G=4, V=Vsp=1280


def _build_host(inputs):
    import ml_dtypes
    bf = ml_dtypes.bfloat16
    f32 = np.float32
    d = {}

    # ---- FC head ----
    xT = np.ascontiguousarray(np.asarray(inputs["x"], f32).T)  # [2048, 256]
    d["xTp_full"] = xT  # sliced + packed per core in kernel()
    fc1wT = np.ascontiguousarray(np.asarray(inputs["fc1_w"], f32).T)  # [2048, 512]
    d["fc1w"] = np.ascontiguousarray(
        fc1wT.reshape(16, 128, 512).transpose(1, 0, 2).reshape(128, 16 * 512)
    ).astype(bf)
    fc2wT = np.ascontiguousarray(np.asarray(inputs["fc2_w"], f32).T)  # [512, 5120]
    f2 = fc2wT.reshape(4, 128, 5120)
    for mc in range(4):
        d[f"fc2w{mc}"] = np.ascontiguousarray(
            f2[:, :, mc * 1280:(mc + 1) * 1280].transpose(1, 0, 2).reshape(128, 4 * 1280)
        ).astype(bf)
    smalls = np.zeros((128, 100), f32)
    for j in range(2):
        smalls[j * 64:(j + 1) * 64, 0:64] += np.eye(64, dtype=f32)
    for j in range(4):
        smalls[j * 32:(j + 1) * 32, 64:96] += np.eye(32, dtype=f32)
    smalls[:, 96:100] = np.asarray(inputs["fc1_b"], f32).reshape(4, 128).T
    d["smalls"] = smalls

    # ---- L matrices ----
    L1 = _dense_L(inputs["L1_rows"], inputs["L1_cols"], inputs["L1_vals"], 320)
    L2 = _dense_L(inputs["L2_rows"], inputs["L2_cols"], inputs["L2_vals"], 1280)
    U1 = np.repeat(np.eye(80, dtype=f32), 4, axis=0)    # [320, 80]
    U2 = np.repeat(np.eye(320, dtype=f32), 4, axis=0)   # [1280, 320]
    LU0 = (L1 @ U1).T                                   # [80, 320]
    LLU0 = 2.0 * (L1 @ (L1 @ U1)).T
    d["LU0p"] = _pad_rows(np.concatenate([LU0, LLU0], axis=1), 128).astype(bf)
    LT1 = _stiles(L1.T)                                 # [128, 3*320]
    LL1 = _stiles(2.0 * (L1 @ L1).T)
    d["LT1p"] = np.concatenate([LT1, LL1], axis=1).astype(bf)
    LU2 = _stiles((L2 @ U2).T)                          # [128, 3*1280]
    LLU2 = _stiles(2.0 * (L2 @ (L2 @ U2)).T)
    d["LU2p"] = np.concatenate([LU2, LLU2], axis=1).astype(bf)
    d["LT2"] = _stiles(L2.T).astype(bf)                 # [128, 10*1280]
    d["LL2"] = _stiles(2.0 * (L2 @ L2).T).astype(bf)

    # ---- Chebyshev linear weight blocks ----
    blks = []
    offs = {}

    def add(nm, arr):
        offs[nm] = sum(b.shape[1] for b in blks)
        blks.append(arr)

    for li, (cfg, wn) in enumerate(zip(CFG, ["cl0_w", "cl1_w", "cl2_w"])):
        W0, W1, W2 = _split_W(inputs[wn])
        A = W0 - W2
        add(f"B{li}", _wbd(W1, cfg.G, cfg.Fin, cfg.Fout))
        add(f"C{li}", _wbd(W2, cfg.G, cfg.Fin, cfg.Fout))
        if cfg.name == "c1":
            for dl in range(2):
                M = np.zeros((128, 128), f32)
                for j in range(2):
                    M[j * 64:(j + 1) * 64,
                      (2 * dl + j) * 32:(2 * dl + j + 1) * 32] = A.T
                add(f"A1_{dl}", M)
        else:
            add(f"A{li}", _wbd(A, cfg.G, cfg.Fin, cfg.Fout))
    W0, W1, W2 = _split_W(inputs["cl3_w"])
    add("B3", _wbd(W1, 4, 32, 3))
    add("C3", _wbd(W2, 4, 32, 3))
    add("A3", _wbd(W0 - W2, 4, 32, 3))
    d["wblk"] = np.concatenate(blks, axis=1).astype(bf)
    d["_woffs"] = offs  # not uploaded

    for i, (g, b) in enumerate([("bn0_g", "bn0_b"), ("bn1_g", "bn1_b"),
                                ("bn2_g", "bn2_b")]):
        gb = np.concatenate([np.asarray(inputs[g], f32),
                             np.asarray(inputs[b], f32)])
        d[f"gb{i}"] = np.ascontiguousarray(gb[None, :])  # [1, 2F]
    b3 = np.asarray(inputs["cl3_b"], f32)
    d["b3r"] = np.ascontiguousarray(np.tile(b3, 160)[None, :])  # [1, 480]
    return d


def _build_nc(woffs):
    import sys
    for p in ("/opt/trn_rl_repo", "/opt/trn_rl_repo/concourse"):
        if p not in sys.path:
            sys.path.insert(0, p)
    import concourse.bass as bass  # noqa
    import concourse.mybir as mybir
    import concourse.tile as tile
    from concourse import bacc
    from concourse.masks import make_identity

    f32 = mybir.dt.float32
    bf16 = mybir.dt.bfloat16
    AF = mybir.ActivationFunctionType
    ALU = mybir.AluOpType

    nc = bacc.Bacc(None, target_bir_lowering=False)

    xTp = nc.dram_tensor("xTp", [128, 16 * BL], bf16, kind="ExternalInput")
    smalls_d = nc.dram_tensor("smalls", [128, 100], f32, kind="ExternalInput")
    fc1w_d = nc.dram_tensor("fc1w", [128, 16 * 512], bf16, kind="ExternalInput")
    fc2w_d = [nc.dram_tensor(f"fc2w{mc}", [128, 4 * 1280], bf16,
                             kind="ExternalInput") for mc in range(4)]
    wblk_d = nc.dram_tensor("wblk", [128, 1188], bf16, kind="ExternalInput")
    LU0p_d = nc.dram_tensor("LU0p", [128, 640], bf16, kind="ExternalInput")
    LT1p_d = nc.dram_tensor("LT1p", [128, 1920], bf16, kind="ExternalInput")
    LU2p_d = nc.dram_tensor("LU2p", [128, 7680], bf16, kind="ExternalInput")
    LT2_d = nc.dram_tensor("LT2", [128, 12800], bf16, kind="ExternalInput")
    LL2_d = nc.dram_tensor("LL2", [128, 12800], bf16, kind="ExternalInput")
    gbs_d = [nc.dram_tensor(f"gb{i}", [1, 2 * F], f32, kind="ExternalInput")
             for i, F in enumerate([64, 32, 32])]
    b3r_d = nc.dram_tensor("b3r", [1, 480], f32, kind="ExternalInput")
    ydram = nc.dram_tensor("y", [128, 960], f32, kind="ExternalOutput")

    with tile.TileContext(nc) as tc:
        with (
            tc.tile_pool(name="wpool", bufs=1) as wpool,
            tc.tile_pool(name="actp", bufs=1) as actp,
            tc.tile_pool(name="misc", bufs=1) as miscp,
            tc.tile_pool(name="pslin", bufs=2, space="PSUM") as pslin,
            tc.tile_pool(name="psW", bufs=3, space="PSUM") as psW,
            tc.tile_pool(name="dram", bufs=1, space="DRAM") as dramp,
        ):
            # ================= SBUF tiles =================
            W1 = wpool.tile([128, 20480], bf16, tag="W1")      # fc2w
            W2 = wpool.tile([128, 8192], bf16, tag="W2")       # fc1w
            LU2p = wpool.tile([128, 7680], bf16, tag="LU2p2")
            LT2 = wpool.tile([128, 12800], bf16, tag="LT2")
            LL2 = wpool.tile([128, 12800], bf16, tag="LL2")
            wblk = wpool.tile([128, 1188], bf16, tag="wblk")
            LU0p = wpool.tile([128, 640], bf16, tag="LU0p")
            LT1p = wpool.tile([128, 1920], bf16, tag="LT1p")
            smalls = wpool.tile([128, 100], f32, tag="smalls")
            xT = wpool.tile([128, 16 * BL], bf16, tag="xT")
            gb_sb = [wpool.tile([1, 2 * F], f32, tag=f"gb{i}",
                                name=f"gb{i}")
                     for i, F in enumerate([64, 32, 32])]
            b3r = wpool.tile([1, 480], f32, tag="b3r")

            def WB(nm, w):
                return wblk[:, woffs[nm]:woffs[nm] + w]

            # ---- DMA issue order == consumption order (SP queue) ----
            nc.sync.dma_start(xT[:], xTp[:])
            nc.sync.dma_start(smalls[:], smalls_d[:])
            for k in range(4):
                nc.sync.dma_start(W2[:, k * 2048:(k + 1) * 2048],
                                  fc1w_d[:, k * 2048:(k + 1) * 2048])
            for mc in range(4):
                nc.sync.dma_start(W1[:, mc * 5120:(mc + 1) * 5120], fc2w_d[mc][:])
            nc.sync.dma_start(wblk[:], wblk_d[:])
            nc.sync.dma_start(LU0p[:], LU0p_d[:])
            nc.sync.dma_start(LT1p[:], LT1p_d[:])
            for i in range(3):
                nc.sync.dma_start(gb_sb[i][:], gbs_d[i][:])
            nc.sync.dma_start(b3r[:], b3r_d[:])
            # Late-use loads are emitted at compute milestones via late_load()
            # (a dummy gate write gives each chunk DMA a data dependency, so
            # the transfers never sit in the DMA_ENGINES queue ahead of the
            # tiny BN-collective DMAs).

            def late_load(dst, dsrc, nchunks, csz, dep):
                for k in range(nchunks):
                    nc.vector.tensor_copy(dst[0:1, k * csz:k * csz + 1], dep)
                    nc.sync.dma_start(dst[:, k * csz:(k + 1) * csz],
                                      dsrc[:, k * csz:(k + 1) * csz])

            # ---- constants / act-table warm ----
            eps_t = miscp.tile([1, 1], f32, tag="eps")
            nc.gpsimd.memset(eps_t[:], EPS)
            ones8 = miscp.tile([8, 1], f32, tag="ones8")
            nc.gpsimd.memset(ones8[:], 1.0)
            onesr = miscp.tile([1, 128], bf16, tag="onesr")
            nc.gpsimd.memset(onesr[:], 1.0)
            b3rb = miscp.tile([1, 480], bf16, tag="b3rb")
            nc.vector.tensor_copy(b3rb[:], b3r[:])
            ident_f = miscp.tile([128, 128], f32, tag="identf")
            make_identity(nc, ident_f[:])
            warm = miscp.tile([1, 4], f32, tag="warm")
            nc.gpsimd.memset(warm[:], 1.0)
            nc.scalar.activation(warm[:, 0:1], warm[:, 0:1], AF.Relu)
            nc.scalar.activation(warm[:, 1:2], warm[:, 1:2], AF.Copy)
            nc.scalar.activation(warm[:, 2:3], warm[:, 2:3], AF.Sqrt)

            # round-robin copy engines for PSUM->SBUF evacuation
            cp_state = [0]

            def cp(dst, src, eng=None):
                e = cp_state[0] % 2 if eng is None else eng
                cp_state[0] += 1
                if e == 0:
                    nc.scalar.activation(dst, src, AF.Copy)
                else:
                    nc.vector.tensor_copy(dst, src)

            # ================= FC head (bf16) =================
            # kt-outer so PE chases the chunked fc1w DMA; each mt group gets
            # its own PSUM bank (one open accumulation group per bank)
            h1T = miscp.tile([128, 4 * BL], bf16, tag="h1T")
            ps1a = psW.tile([128, 1024], f32, tag="big", name="ps1a")
            ps1b = psW.tile([128, 1024], f32, tag="big", name="ps1b")
            mtsl = [(ps1a, 0), (ps1a, 512), (ps1b, 0), (ps1b, 512)]
            for kt in range(16):
                for mt in range(4):
                    pt_, c0_ = mtsl[mt]
                    nc.tensor.matmul(
                        pt_[:, c0_:c0_ + BL],
                        W2[:, kt * 512 + mt * 128: kt * 512 + (mt + 1) * 128],
                        xT[:, kt * BL:(kt + 1) * BL],
                        start=(kt == 0), stop=(kt == 15),
                        skip_group_check=True)
            for mt in range(4):
                pt_, c0_ = mtsl[mt]
                nc.scalar.activation(
                    h1T[:, mt * BL:(mt + 1) * BL], pt_[:, c0_:c0_ + BL],
                    AF.Relu, bias=smalls[:, 96 + mt:97 + mt])

            XF0 = actp.tile([128, 16 * 80], bf16, tag="XF0")
            for mc in range(4):
                ps2 = psW.tile([128, 1024], f32, tag="big")
                for mi in range(10):
                    for kt in range(4):
                        nc.tensor.matmul(
                            ps2[:, mi * BL:(mi + 1) * BL],
                            W1[:, mc * 5120 + kt * 1280 + mi * 128:
                               mc * 5120 + kt * 1280 + (mi + 1) * 128],
                            h1T[:, kt * BL:(kt + 1) * BL],
                            start=(kt == 0), stop=(kt == 3),
                            skip_group_check=True)
                # psum [(v0%2)*64+f, b] -> XF0[(b%2)*64+f, (b//2)*80 + v0]
                src4 = ps2[:, 0:320].rearrange("p (i g j) -> p i g j", g=16, j=2)
                dst4 = XF0[:].rearrange("p (g u q) -> p g u q", u=40, q=2)
                for p0 in range(2):
                    for j in range(2):
                        nc.scalar.activation(
                            dst4[j * 64:(j + 1) * 64, :,
                                 mc * 10:(mc + 1) * 10, p0]
                            .rearrange("p g i -> p i g"),
                            src4[p0 * 64:(p0 + 1) * 64, :, :, j],
                            AF.Copy)

            # ================= cheby layers c0-c2 (F-layout) =================
            XF_cur = XF0

            for li, cfg in enumerate(CFG):
                V, Vin, F = cfg.V, cfg.Vin, cfg.Fout
                BF, nS, nGp = cfg.BF, cfg.nS, cfg.nGp
                # L-matrix rhs tiles: [128, nS*V (L-part) | nS*V (2L^2-part)]
                if cfg.name == "c0":
                    Lr, lw = LU0p, 320
                elif cfg.name == "c1":
                    Lr, lw = LT1p, 320
                else:
                    Lr, lw = LU2p, 1280

                # ---- B/C linears into source-vertex space ----
                XB = actp.tile([128, 3072], bf16, tag="XB",
                               name="XB")[:, :nS * BF]
                XC = actp.tile([128, 3072], bf16, tag="XC",
                               name="XC")[:, :nS * BF]
                gpack = max(1, 512 // cfg.GF)
                for s in range(nS):
                    ssz = cfg.sps(s)
                    for dst, wnm in ((XB, f"B{li}"), (XC, f"C{li}")):
                        Wt = WB(wnm, cfg.GF)
                        for g0 in range(0, cfg.nG, gpack):
                            gn = min(gpack, cfg.nG - g0)
                            pc = pslin.tile([128, 512], f32, tag="lin")
                            for gi in range(gn):
                                g = g0 + gi
                                nc.tensor.matmul(
                                    pc[:ssz, gi * cfg.GF:(gi + 1) * cfg.GF],
                                    XF_cur[:, g * Vin + s * 128:
                                           g * Vin + s * 128 + ssz],
                                    Wt, start=True, stop=True,
                                    skip_group_check=True)
                            cp(dst[:ssz, s * BF + g0 * cfg.GF:
                                   s * BF + (g0 + gn) * cfg.GF],
                               pc[:ssz, :gn * cfg.GF])

                if li == 0:
                    # pace LU2p load: gated on c0's first B-linear output so
                    # its transfers neither delay fc2w nor collide with the
                    # BN0 collective's DMA window
                    late_load(LU2p, LU2p_d, 4, 1920, XB[0:1, 0:1])

                # ---- transposed spmm + copies + 512-wide stats windows ----
                XFn = actp.tile([128, nGp * V], bf16, tag=f"XFn{li}")
                FD = nGp * V
                nch = FD // 512
                bnst = miscp.tile([128, nch * 6], f32, tag=f"bnst{li}")
                stat_done = [0, 0]  # cols copied, windows emitted

                def emit_stats(done, XFn=XFn, bnst=bnst, sd=stat_done, nch=nch):
                    sd[0] = done
                    while sd[1] < nch and (sd[1] + 1) * 512 <= sd[0]:
                        ci = sd[1]
                        nc.vector.bn_stats(
                            bnst[:, ci * 6:(ci + 1) * 6],
                            XFn[:, ci * 512:(ci + 1) * 512])
                        sd[1] += 1

                def spmm_group(ps, pbase, gp, w0, wcw):
                    """Accumulate output block (gp, w0:w0+wcw) into ps cols
                    pbase:pbase+wcw (wcw <= 512)."""
                    for half, XS in ((0, XB), (1, XC)):
                        for s in range(nS):
                            ssz = cfg.sps(s)
                            nc.tensor.matmul(
                                ps[:, pbase:pbase + wcw],
                                XS[:ssz, s * BF + gp * 128:
                                   s * BF + (gp + 1) * 128],
                                Lr[:ssz, half * nS * lw + s * lw + w0:
                                   half * nS * lw + s * lw + w0 + wcw],
                                start=(half == 0 and s == 0), stop=False,
                                skip_group_check=True)
                    if cfg.name == "c1":
                        for dl in range(2):
                            nc.tensor.matmul(
                                ps[:, pbase:pbase + wcw],
                                WB(f"A1_{dl}", 128),
                                XF_cur[:, (2 * gp + dl) * Vin + w0:
                                       (2 * gp + dl) * Vin + w0 + wcw],
                                start=False, stop=(dl == 1),
                                skip_group_check=True)
                    else:
                        rhs = XF_cur[:, gp * Vin + w0 // 4:
                                     gp * Vin + w0 // 4 + wcw // 4]
                        rhs = rhs.broadcast_to([128, wcw // 4, 4])
                        nc.tensor.matmul(
                            ps[:, pbase:pbase + wcw], WB(f"A{li}", 128), rhs,
                            start=False, stop=True, skip_group_check=True)

                if V <= 512:
                    # pack 2 gp-blocks per PSUM tile (bank-aligned at 512)
                    for gpp in range(0, nGp, 2):
                        ps = psW.tile([128, 1024], f32, tag="big")
                        for gi in range(2):
                            spmm_group(ps, gi * 512, gpp + gi, 0, V)
                        cp(XFn[:, gpp * V:(gpp + 2) * V]
                           .rearrange("p (u w) -> p u w", w=V),
                           ps[:].rearrange("p (u w) -> p u w", w=512)[:, :, :V],
                           eng=0 if gpp + 2 >= nGp else None)
                        emit_stats((gpp + 2) * V)
                else:
                    # c2: V=1280 per gp -> tiles of 1024 + 256
                    for gp in range(nGp):
                        for w0 in (0, 1024):
                            wcw = min(1024, V - w0)
                            ps = psW.tile([128, 1024], f32, tag="big")
                            for nk in range(0, wcw, 512):
                                sub = min(512, wcw - nk)
                                spmm_group(ps, nk, gp, w0 + nk, sub)
                            cp(XFn[:, gp * V + w0: gp * V + w0 + wcw],
                               ps[:, :wcw],
                               eng=0 if gp == nGp - 1 else None)
                            emit_stats(gp * V + w0 + wcw)

                # ---- BN: partial sums -> AllGather -> scale/shift ----
                n_g = float(B * V)
                aggr = miscp.tile([128, 2], f32, tag="aggr")
                nc.vector.bn_aggr(
                    aggr[:], bnst[:].rearrange("p (c s) -> p c s", s=6))
                part = miscp.tile([128, 2], f32, tag="part")
                nc.vector.tensor_tensor(
                    out=part[:, 1:2], in0=aggr[:, 0:1], in1=aggr[:, 0:1],
                    op=ALU.mult)
                nc.vector.tensor_tensor(
                    out=part[:, 1:2], in0=part[:, 1:2], in1=aggr[:, 1:2],
                    op=ALU.add)
                fscl = float(FD) / n_g
                nc.vector.tensor_scalar_mul(part[:, 1:2], part[:, 1:2], fscl)
                nc.vector.tensor_scalar_mul(part[:, 0:1], aggr[:, 0:1], fscl)
                sel = smalls[:, 0:64] if F == 64 else smalls[:, 64:96]
                pst = pslin.tile([128, 512], f32, tag="lin")
                nc.tensor.matmul(pst[:1, :F], part[:, 0:1], sel,
                                 start=True, stop=True, skip_group_check=True)
                nc.tensor.matmul(pst[:1, F:2 * F], part[:, 1:2], sel,
                                 start=True, stop=True, skip_group_check=True)
                stats_l = miscp.tile([1, 2 * F], f32, tag="statl")
                nc.vector.tensor_copy(stats_l[:], pst[:1, :2 * F])
                bin_ = dramp.tile([1, 2 * F], f32, tag=f"arin{li}")
                bout = dramp.tile([NCORES, 2 * F], f32, tag=f"arout{li}")
                nc.sync.dma_start(bin_[:], stats_l[:])
                nc.gpsimd.collective_compute(
                    "AllGather", ALU.bypass,
                    replica_groups=[list(range(NCORES))],
                    ins=[bin_.opt()], outs=[bout.opt()])
                statg8 = miscp.tile([NCORES, 2 * F], f32, tag="statg8")
                nc.sync.dma_start(statg8[:], bout[:])
                psg = pslin.tile([128, 512], f32, tag="lin")
                nc.tensor.matmul(psg[:1, :2 * F], ones8[:], statg8[:],
                                 start=True, stop=True, skip_group_check=True)
                # s,t from global [mu | Ex^2] (psg)
                st = miscp.tile([1, 2 * F], f32, tag="st")
                tmp = miscp.tile([1, 2 * F], f32, tag="sttmp")
                mu2 = miscp.tile([1, F], f32, tag="mu2")
                nc.vector.tensor_copy(tmp[:, :2 * F], psg[:1, :2 * F])
                nc.vector.tensor_tensor(out=mu2[:], in0=tmp[:, 0:F],
                                        in1=tmp[:, 0:F], op=ALU.mult)
                nc.vector.tensor_tensor(out=tmp[:, F:2 * F],
                                        in0=tmp[:, F:2 * F],
                                        in1=mu2[:], op=ALU.subtract)
                nc.scalar.activation(tmp[:, F:2 * F], tmp[:, F:2 * F],
                                     AF.Sqrt, bias=eps_t[:])
                nc.vector.reciprocal(tmp[:, F:2 * F], tmp[:, F:2 * F])
                nc.vector.tensor_tensor(out=st[:, 0:F],
                                        in0=tmp[:, F:2 * F],
                                        in1=gb_sb[li][:, 0:F], op=ALU.mult)
                nc.vector.tensor_tensor(out=mu2[:], in0=tmp[:, 0:F],
                                        in1=st[:, 0:F], op=ALU.mult)
                nc.vector.tensor_tensor(out=st[:, F:2 * F],
                                        in0=gb_sb[li][:, F:2 * F],
                                        in1=mu2[:], op=ALU.subtract)
                pss = pslin.tile([128, 512], f32, tag="lin", name="pss")
                nc.tensor.transpose(pss[:2 * F, 0:1], st[:],
                                    ident_f[:1, :1])
                stc = miscp.tile([128, 2], f32, tag=f"stc{li}")
                for j in range(cfg.Gp):
                    nc.vector.tensor_copy(stc[j * F:(j + 1) * F, 0:1],
                                          pss[:F, 0:1])
                    nc.vector.tensor_copy(stc[j * F:(j + 1) * F, 1:2],
                                          pss[F:2 * F, 0:1])
                # chunked scale/shift + relu, split across Act/DVE/Pool
                # (contiguous g-aligned chunks: subtile deps track them)
                def relu_chunk(ap, eng):
                    if eng == 1:
                        nc.vector.tensor_scalar(ap, ap, stc[:, 0:1],
                                                stc[:, 1:2], ALU.mult, ALU.add)
                        nc.vector.tensor_scalar_max(ap, ap, 0.0)
                    elif eng == 2:
                        nc.gpsimd.tensor_scalar(ap, ap, stc[:, 0:1],
                                                stc[:, 1:2], ALU.mult, ALU.add)
                        nc.gpsimd.tensor_scalar_max(ap, ap, 0.0)
                    else:
                        nc.scalar.activation(ap, ap, AF.Relu,
                                             scale=stc[:, 0:1],
                                             bias=stc[:, 1:2])

                csz = FD // 8
                # engine shares ~ inverse throughput: Act 4, DVE 3, Pool 1
                sched = [0, 1, 0, 1, 2, 0, 1, 0]
                for rc in range(8):
                    relu_chunk(XFn[:, rc * csz:(rc + 1) * csz], sched[rc])
                if li == 0:
                    late_load(LT2, LT2_d, 8, 1600, XFn[0:1, 0:1])
                elif li == 1:
                    late_load(LL2, LL2_d, 8, 1600, XFn[0:1, 0:1])
                XF_cur = XFn

            # ================= c3 (V-layout) + output =================
            # XF_cur = XF2 [128 (j4,c32), 8*1280], G=4, nG=8.
            # Per-s pipeline: relu chunk s -> B/C linears of source tile s ->
            # contributions of s accumulated into 2 persistent PSUM tiles
            # (t 0-4 and t 5-9), so the spmm overlaps the relu chunks.
            XB3 = actp.tile([128, 3072], bf16, tag="XB",
                            name="XB3")[:, :960]
            XC3 = actp.tile([128, 3072], bf16, tag="XC",
                            name="XC3")[:, :960]
            for s in range(10):
                pc = pslin.tile([128, 512], f32, tag="lin")
                for dst, wnm, off in ((XB3, "B3", 0), (XC3, "C3", 96)):
                    Wt = WB(wnm, 12)
                    for g in range(8):
                        nc.tensor.matmul(
                            pc[:, off + g * 12: off + (g + 1) * 12],
                            XF_cur[:, g * 1280 + s * 128:
                                   g * 1280 + (s + 1) * 128],
                            Wt, start=True, stop=True, skip_group_check=True)
                cp(XB3[:, s * 96:(s + 1) * 96], pc[:, 0:96])
                cp(XC3[:, s * 96:(s + 1) * 96], pc[:, 96:192])
            ysb = miscp.tile([128, 960], f32, tag="ysb")
            for t in range(10):
                pv = psW.tile([128, 1024], f32, tag="big")
                for s in range(10):
                    nc.tensor.matmul(
                        pv[:, 0:96],
                        LT2[:, s * 1280 + t * 128:s * 1280 + (t + 1) * 128],
                        XB3[:, s * 96:(s + 1) * 96],
                        start=(s == 0), stop=False, skip_group_check=True)
                for s in range(10):
                    nc.tensor.matmul(
                        pv[:, 0:96],
                        LL2[:, s * 1280 + t * 128:s * 1280 + (t + 1) * 128],
                        XC3[:, s * 96:(s + 1) * 96],
                        start=False, stop=False, skip_group_check=True)
                for g in range(8):
                    nc.tensor.matmul(
                        pv[:, g * 12:(g + 1) * 12],
                        XF_cur[:, g * 1280 + t * 128:g * 1280 + (t + 1) * 128],
                        WB("A3", 12),
                        start=False, stop=False, skip_group_check=True)
                nc.tensor.matmul(
                    pv[:, 0:96], onesr[:, :128], b3rb[:, 0:96],
                    start=False, stop=True, skip_group_check=True)
                cp(ysb[:, t * 96:(t + 1) * 96], pv[:, 0:96])
            nc.sync.dma_start(ydram[:, 0:480], ysb[:, 0:480])
            nc.sync.dma_start(ydram[:, 480:960], ysb[:, 480:960])

    nc.compile()
    return nc


def kernel(**inputs):
    import sys
    for p in ("/opt/trn_rl_repo", "/opt/trn_rl_repo/concourse"):
        if p not in sys.path:
            sys.path.insert(0, p)
    from concourse.bass_utils import run_bass_kernel_spmd
    import ml_dtypes

    host = _build_host(inputs)
    woffs = host.pop("_woffs")
    xT_full = host.pop("xTp_full")

    key = ("nc",)
    if key not in _CACHE:
        _CACHE[key] = _build_nc(woffs)
    nc = _CACHE[key]

    in_maps = []
    for c in range(NCORES):
        m = dict(host)
        xc = xT_full[:, c * BL:(c + 1) * BL]  # [2048, 32]
        m["xTp"] = np.ascontiguousarray(
            xc.reshape(16, 128, BL).transpose(1, 0, 2).reshape(128, 16 * BL)
        ).astype(ml_dtypes.bfloat16)
        in_maps.append(m)
    res = run_bass_kernel_spmd(nc, in_maps, core_ids=list(range(NCORES)))
    outs = []
    for c in range(NCORES):
        y = res.results[c]["y"].astype(np.float32)  # [128, 960]
        outs.append(y.reshape(128, 10, BL, 3).transpose(2, 1, 0, 3)
                    .reshape(BL, 1280, 3))
    return np.concatenate(outs, axis=0)


if __name__ == "__main__":
    import reference as R
    inp = R.setup_inputs()
    inp = {k: np.asarray(v) for k, v in inp.items()}
    act = kernel(**inp)
    exp = np.asarray(R.reference(**inp))
    err = np.linalg.norm(act - exp) / np.linalg.norm(exp)
    print("Relative error:", err)


# revision 23
# speedup vs baseline: 1.9668x; 1.0098x over previous
"""Trainium2 Bass kernel for nn_Graph_CNN_Feat_Mesh (Chebyshev GNN decoder).

Strategy (per-core, data-parallel over batch B=256 -> 32/core):
  - All spmms are dense matmuls on the tensor engine (PE) in bf16.
    For K=3 Chebyshev conv:  y = A(x) + L @ B(x) + (2 L^2) @ C(x)
    with A = W0-W2, B = W1, C = W2 applied per-vertex in feature space.
    For up4-preceded layers, replication is folded into the host-side
    matrices:  y = A(x_up) + (L U) @ B(x320) + (2 L^2 U) @ C(x320),
    so both spmms contract over the small pre-upsample vertex space.
  - Layers c0-c2 run the spmm TRANSPOSED (lhsT = feature tiles, rhs = L
    tiles), emitting the next layer's packed F-layout directly: no
    back-transposes.  The A-term accumulates into the same PSUM with a
    stride-0 broadcast rhs for the up4 replication.
  - BatchNorm (training mode, global batch stats) is exact: per-core
    partial sums are AllGather'd across the 8 cores (cheaper than
    AllReduce) and summed locally; scale/shift+relu is applied in column
    chunks feeding the next layer's matmuls incrementally.
  - FC head runs in bf16 with fp32 PSUM accumulation; weight DMAs are
    issued in consumption order and big late-use matrices (L2, 2*L2^2)
    alias the FC weight SBUF space (chunked so the tiny BN collective
    DMAs never queue behind a long transfer).
"""

import numpy as np

B = 256
NCORES = 8
BL = B // NCORES  # 32
EPS = 1e-5

_CACHE = {}


def _split_W(W):
    W = np.asarray(W, np.float32)
    return W[:, 0::3], W[:, 1::3], W[:, 2::3]


def _dense_L(rows, cols, vals, V):
    L = np.zeros((V, V), np.float32)
    np.add.at(L, (np.asarray(rows), np.asarray(cols)), np.asarray(vals, np.float32))
    return L


def _pad_rows(a, m):
    if a.shape[0] % m == 0:
        return a
    p = m - a.shape[0] % m
    return np.concatenate([a, np.zeros((p,) + a.shape[1:], a.dtype)], 0)


def _stiles(a):
    """[U, V] -> [128, nS*V] with s-tiles of 128 source rows side by side."""
    a = _pad_rows(np.ascontiguousarray(a), 128)
    nS = a.shape[0] // 128
    return np.concatenate([a[s * 128:(s + 1) * 128, :] for s in range(nS)], axis=1)


def _wbd(M, G, Fin, Fout):
    """Block-diagonal weight [128, G*Fout]; block j holds M.T ([Fin, Fout])."""
    out = np.zeros((128, G * Fout), np.float32)
    for j in range(G):
        out[j * Fin:(j + 1) * Fin, j * Fout:(j + 1) * Fout] = M.T
    return out


class _LCfg:
    def __init__(self, name, Vin, Vsp, V, Fin, Fout, up4):
        self.name = name
        self.Vin = Vin            # per-g input column span of XF
        self.Vsp = Vsp            # source vertex space of B/C linears
        self.V = V                # output vertex count
        self.Fin = Fin
        self.Fout = Fout
        self.G = 128 // Fin       # input batch packs
        self.nG = BL // self.G
        self.GF = self.G * Fout
        self.Gp = 128 // Fout     # output batch packs
        self.BF = BL * Fout
        self.nGp = self.BF // 128  # output 128-col blocks
        self.nS = (Vsp + 127) // 128
        self.up4 = up4

    def sps(self, s):
        return min(128, self.Vsp - s * 128)


CFG = [
    _LCfg("c0", 80, 80, 320, 64, 64, True),
    _LCfg("c1", 320, 320, 320, 64, 32, False),
    _LCfg("c2", 320, 320, 1280, 32, 32, True),
]
# c3 (V-layout output layer): Fin=32, Fout=3, G=4, V=Vsp=1280


def _build_host(inputs):
    import ml_dtypes
    bf = ml_dtypes.bfloat16
    f32 = np.float32
    d = {}

    # ---- FC head ----
    xT = np.ascontiguousarray(np.asarray(inputs["x"], f32).T)  # [2048, 256]
    d["xTp_full"] = xT  # sliced + packed per core in kernel()
    fc1wT = np.ascontiguousarray(np.asarray(inputs["fc1_w"], f32).T)  # [2048, 512]
    d["fc1w"] = np.ascontiguousarray(
        fc1wT.reshape(16, 128, 512).transpose(1, 0, 2).reshape(128, 16 * 512)
    ).astype(bf)
    fc2wT = np.ascontiguousarray(np.asarray(inputs["fc2_w"], f32).T)  # [512, 5120]
    f2 = fc2wT.reshape(4, 128, 5120)
    for mc in range(4):
        d[f"fc2w{mc}"] = np.ascontiguousarray(
            f2[:, :, mc * 1280:(mc + 1) * 1280].transpose(1, 0, 2).reshape(128, 4 * 1280)
        ).astype(bf)
    smalls = np.zeros((128, 100), f32)
    for j in range(2):
        smalls[j * 64:(j + 1) * 64, 0:64] += np.eye(64, dtype=f32)
    for j in range(4):
        smalls[j * 32:(j + 1) * 32, 64:96] += np.eye(32, dtype=f32)
    smalls[:, 96:100] = np.asarray(inputs["fc1_b"], f32).reshape(4, 128).T
    d["smalls"] = smalls

    # ---- L matrices ----
    L1 = _dense_L(inputs["L1_rows"], inputs["L1_cols"], inputs["L1_vals"], 320)
    L2 = _dense_L(inputs["L2_rows"], inputs["L2_cols"], inputs["L2_vals"], 1280)
    U1 = np.repeat(np.eye(80, dtype=f32), 4, axis=0)    # [320, 80]
    U2 = np.repeat(np.eye(320, dtype=f32), 4, axis=0)   # [1280, 320]
    LU0 = (L1 @ U1).T                                   # [80, 320]
    LLU0 = 2.0 * (L1 @ (L1 @ U1)).T
    d["LU0p"] = _pad_rows(np.concatenate([LU0, LLU0], axis=1), 128).astype(bf)
    # [L ; 2L^2] stacked vertically -> 5 full 128-row K-tiles
    d["LT1p"] = _stiles(np.concatenate(
        [L1.T, 2.0 * (L1 @ L1).T], axis=0)).astype(bf)      # [128, 5*320]
    d["LU2p"] = _stiles(np.concatenate(
        [(L2 @ U2).T, 2.0 * (L2 @ (L2 @ U2)).T], axis=0)).astype(bf)  # [128, 5*1280]
    d["LT2"] = _stiles(L2.T).astype(bf)                 # [128, 10*1280]
    d["LL2"] = _stiles(2.0 * (L2 @ L2).T).astype(bf)

    # ---- Chebyshev linear weight blocks ----
    blks = []
    offs = {}

    def add(nm, arr):
        offs[nm] = sum(b.shape[1] for b in blks)
        blks.append(arr)

    for li, (cfg, wn) in enumerate(zip(CFG, ["cl0_w", "cl1_w", "cl2_w"])):
        W0, W1, W2 = _split_W(inputs[wn])
        A = W0 - W2
        add(f"B{li}", _wbd(W1, cfg.G, cfg.Fin, cfg.Fout))
        add(f"C{li}", _wbd(W2, cfg.G, cfg.Fin, cfg.Fout))
        if cfg.name == "c1":
            for dl in range(2):
                M = np.zeros((128, 128), f32)
                for j in range(2):
                    M[j * 64:(j + 1) * 64,
                      (2 * dl + j) * 32:(2 * dl + j + 1) * 32] = A.T
                add(f"A1_{dl}", M)
        else:
            add(f"A{li}", _wbd(A, cfg.G, cfg.Fin, cfg.Fout))
    W0, W1, W2 = _split_W(inputs["cl3_w"])
    add("B3", _wbd(W1, 4, 32, 3))
    add("C3", _wbd(W2, 4, 32, 3))
    add("A3", _wbd(W0 - W2, 4, 32, 3))
    d["wblk"] = np.concatenate(blks, axis=1).astype(bf)
    d["_woffs"] = offs  # not uploaded

    for i, (g, b) in enumerate([("bn0_g", "bn0_b"), ("bn1_g", "bn1_b"),
                                ("bn2_g", "bn2_b")]):
        gb = np.concatenate([np.asarray(inputs[g], f32),
                             np.asarray(inputs[b], f32)])
        d[f"gb{i}"] = np.ascontiguousarray(gb[None, :])  # [1, 2F]
    b3 = np.asarray(inputs["cl3_b"], f32)
    d["b3r"] = np.ascontiguousarray(np.tile(b3, 160)[None, :])  # [1, 480]
    return d


def _build_nc(woffs):
    import sys
    for p in ("/opt/trn_rl_repo", "/opt/trn_rl_repo/concourse"):
        if p not in sys.path:
            sys.path.insert(0, p)
    import concourse.bass as bass  # noqa
    import concourse.mybir as mybir
    import concourse.tile as tile
    from concourse import bacc
    from concourse.masks import make_identity

    f32 = mybir.dt.float32
    bf16 = mybir.dt.bfloat16
    AF = mybir.ActivationFunctionType
    ALU = mybir.AluOpType

    nc = bacc.Bacc(None, target_bir_lowering=False)

    xTp = nc.dram_tensor("xTp", [128, 16 * BL], bf16, kind="ExternalInput")
    smalls_d = nc.dram_tensor("smalls", [128, 100], f32, kind="ExternalInput")
    fc1w_d = nc.dram_tensor("fc1w", [128, 16 * 512], bf16, kind="ExternalInput")
    fc2w_d = [nc.dram_tensor(f"fc2w{mc}", [128, 4 * 1280], bf16,
                             kind="ExternalInput") for mc in range(4)]
    wblk_d = nc.dram_tensor("wblk", [128, 1188], bf16, kind="ExternalInput")
    LU0p_d = nc.dram_tensor("LU0p", [128, 640], bf16, kind="ExternalInput")
    LT1p_d = nc.dram_tensor("LT1p", [128, 1600], bf16, kind="ExternalInput")
    LU2p_d = nc.dram_tensor("LU2p", [128, 6400], bf16, kind="ExternalInput")
    LT2_d = nc.dram_tensor("LT2", [128, 12800], bf16, kind="ExternalInput")
    LL2_d = nc.dram_tensor("LL2", [128, 12800], bf16, kind="ExternalInput")
    gbs_d = [nc.dram_tensor(f"gb{i}", [1, 2 * F], f32, kind="ExternalInput")
             for i, F in enumerate([64, 32, 32])]
    b3r_d = nc.dram_tensor("b3r", [1, 480], f32, kind="ExternalInput")
    ydram = nc.dram_tensor("y", [128, 960], f32, kind="ExternalOutput")

    with tile.TileContext(nc) as tc:
        with (
            tc.tile_pool(name="wpool", bufs=1) as wpool,
            tc.tile_pool(name="actp", bufs=1) as actp,
            tc.tile_pool(name="misc", bufs=1) as miscp,
            tc.tile_pool(name="pslin", bufs=2, space="PSUM") as pslin,
            tc.tile_pool(name="psW", bufs=3, space="PSUM") as psW,
            tc.tile_pool(name="dram", bufs=1, space="DRAM") as dramp,
        ):
            # ================= SBUF tiles =================
            W1 = wpool.tile([128, 20480], bf16, tag="W1")      # fc2w
            W2 = wpool.tile([128, 8192], bf16, tag="W2")       # fc1w
            LU2p = wpool.tile([128, 6400], bf16, tag="LU2p2")
            LT2 = wpool.tile([128, 12800], bf16, tag="LT2")
            LL2 = wpool.tile([128, 12800], bf16, tag="LL2")
            wblk = wpool.tile([128, 1188], bf16, tag="wblk")
            LU0p = wpool.tile([128, 640], bf16, tag="LU0p")
            LT1p = wpool.tile([128, 1600], bf16, tag="LT1p")
            smalls = wpool.tile([128, 100], f32, tag="smalls")
            xT = wpool.tile([128, 16 * BL], bf16, tag="xT")
            gb_sb = [wpool.tile([1, 2 * F], f32, tag=f"gb{i}",
                                name=f"gb{i}")
                     for i, F in enumerate([64, 32, 32])]
            b3r = wpool.tile([1, 480], f32, tag="b3r")

            def WB(nm, w):
                return wblk[:, woffs[nm]:woffs[nm] + w]

            # ---- DMA issue order == consumption order (SP queue) ----
            nc.sync.dma_start(xT[:], xTp[:])
            nc.sync.dma_start(smalls[:], smalls_d[:])
            for k in range(4):
                nc.sync.dma_start(W2[:, k * 2048:(k + 1) * 2048],
                                  fc1w_d[:, k * 2048:(k + 1) * 2048])
            for mc in range(4):
                nc.sync.dma_start(W1[:, mc * 5120:(mc + 1) * 5120], fc2w_d[mc][:])
            nc.sync.dma_start(wblk[:], wblk_d[:])
            nc.sync.dma_start(LU0p[:], LU0p_d[:])
            nc.sync.dma_start(LT1p[:], LT1p_d[:])
            for i in range(3):
                nc.sync.dma_start(gb_sb[i][:], gbs_d[i][:])
            nc.sync.dma_start(b3r[:], b3r_d[:])
            # Late-use loads are emitted at compute milestones via late_load()
            # (a dummy gate write gives each chunk DMA a data dependency, so
            # the transfers never sit in the DMA_ENGINES queue ahead of the
            # tiny BN-collective DMAs).

            def late_load(dst, dsrc, nchunks, csz, dep):
                for k in range(nchunks):
                    nc.vector.tensor_copy(dst[0:1, k * csz:k * csz + 1], dep)
                    nc.sync.dma_start(dst[:, k * csz:(k + 1) * csz],
                                      dsrc[:, k * csz:(k + 1) * csz])

            # ---- constants / act-table warm ----
            eps_t = miscp.tile([1, 1], f32, tag="eps")
            nc.gpsimd.memset(eps_t[:], EPS)
            ones8 = miscp.tile([8, 1], f32, tag="ones8")
            nc.gpsimd.memset(ones8[:], 1.0)
            onesr = miscp.tile([1, 128], bf16, tag="onesr")
            nc.gpsimd.memset(onesr[:], 1.0)
            b3rb = miscp.tile([1, 480], bf16, tag="b3rb")
            nc.vector.tensor_copy(b3rb[:], b3r[:])
            ident_f = miscp.tile([128, 128], f32, tag="identf")
            make_identity(nc, ident_f[:])
            warm = miscp.tile([1, 4], f32, tag="warm")
            nc.gpsimd.memset(warm[:], 1.0)
            nc.scalar.activation(warm[:, 0:1], warm[:, 0:1], AF.Relu)
            nc.scalar.activation(warm[:, 1:2], warm[:, 1:2], AF.Copy)
            nc.scalar.activation(warm[:, 2:3], warm[:, 2:3], AF.Sqrt)

            # round-robin copy engines for PSUM->SBUF evacuation
            cp_state = [0]

            def cp(dst, src, eng=None):
                e = cp_state[0] % 2 if eng is None else eng
                cp_state[0] += 1
                if e == 0:
                    nc.scalar.activation(dst, src, AF.Copy)
                else:
                    nc.vector.tensor_copy(dst, src)

            # ================= FC head (bf16) =================
            # kt-outer so PE chases the chunked fc1w DMA; each mt group gets
            # its own PSUM bank (one open accumulation group per bank)
            h1T = miscp.tile([128, 4 * BL], bf16, tag="h1T")
            ps1a = psW.tile([128, 1024], f32, tag="big", name="ps1a")
            ps1b = psW.tile([128, 1024], f32, tag="big", name="ps1b")
            mtsl = [(ps1a, 0), (ps1a, 512), (ps1b, 0), (ps1b, 512)]
            for kt in range(16):
                for mt in range(4):
                    pt_, c0_ = mtsl[mt]
                    nc.tensor.matmul(
                        pt_[:, c0_:c0_ + BL],
                        W2[:, kt * 512 + mt * 128: kt * 512 + (mt + 1) * 128],
                        xT[:, kt * BL:(kt + 1) * BL],
                        start=(kt == 0), stop=(kt == 15),
                        skip_group_check=True)
            for mt in range(4):
                pt_, c0_ = mtsl[mt]
                nc.scalar.activation(
                    h1T[:, mt * BL:(mt + 1) * BL], pt_[:, c0_:c0_ + BL],
                    AF.Relu, bias=smalls[:, 96 + mt:97 + mt])

            XF0 = actp.tile([128, 16 * 80], bf16, tag="XF0")
            for mc in range(4):
                ps2 = psW.tile([128, 1024], f32, tag="big")
                for mi in range(10):
                    for kt in range(4):
                        nc.tensor.matmul(
                            ps2[:, mi * BL:(mi + 1) * BL],
                            W1[:, mc * 5120 + kt * 1280 + mi * 128:
                               mc * 5120 + kt * 1280 + (mi + 1) * 128],
                            h1T[:, kt * BL:(kt + 1) * BL],
                            start=(kt == 0), stop=(kt == 3),
                            skip_group_check=True)
                # psum [(v0%2)*64+f, b] -> XF0[(b%2)*64+f, (b//2)*80 + v0]
                src4 = ps2[:, 0:320].rearrange("p (i g j) -> p i g j", g=16, j=2)
                dst4 = XF0[:].rearrange("p (g u q) -> p g u q", u=40, q=2)
                for p0 in range(2):
                    for j in range(2):
                        nc.scalar.activation(
                            dst4[j * 64:(j + 1) * 64, :,
                                 mc * 10:(mc + 1) * 10, p0]
                            .rearrange("p g i -> p i g"),
                            src4[p0 * 64:(p0 + 1) * 64, :, :, j],
                            AF.Copy)

            # ================= cheby layers c0-c2 (F-layout) =================
            XF_cur = XF0

            for li, cfg in enumerate(CFG):
                V, Vin, F = cfg.V, cfg.Vin, cfg.Fout
                BF, nS, nGp = cfg.BF, cfg.nS, cfg.nGp
                # L-matrix rhs tiles: [128, nS*V (L-part) | nS*V (2L^2-part)]
                if cfg.name == "c0":
                    Lr, lw = LU0p, 320
                elif cfg.name == "c1":
                    Lr, lw = LT1p, 320
                else:
                    Lr, lw = LU2p, 1280

                # ---- B/C linears into source-vertex space ----
                XB = actp.tile([128, 5120], bf16, tag="XB",
                               name="XB")
                XC = actp.tile([128, 2048], bf16, tag="XC",
                               name="XC")
                gpack = max(1, 512 // cfg.GF)
                for s in range(nS):
                    ssz = cfg.sps(s)
                    for which, wnm in ((0, f"B{li}"), (1, f"C{li}")):
                        Wt = WB(wnm, cfg.GF)
                        for g0 in range(0, cfg.nG, gpack):
                            gn = min(gpack, cfg.nG - g0)
                            pc = pslin.tile([128, 512], f32, tag="lin")
                            for gi in range(gn):
                                g = g0 + gi
                                nc.tensor.matmul(
                                    pc[:ssz, gi * cfg.GF:(gi + 1) * cfg.GF],
                                    XF_cur[:, g * Vin + s * 128:
                                           g * Vin + s * 128 + ssz],
                                    Wt, start=True, stop=True,
                                    skip_group_check=True)
                            c0_, c1_ = g0 * cfg.GF, (g0 + gn) * cfg.GF
                            cw = c1_ - c0_
                            if li == 0:
                                dst = XB if which == 0 else XC
                                cp(dst[:ssz, s * BF + c0_:s * BF + c1_],
                                   pc[:ssz, :cw])
                            elif which == 0:
                                # B rows -> stacked tiles s (same partitions)
                                cp(XB[:ssz, s * BF + c0_:s * BF + c1_],
                                   pc[:ssz, :cw])
                            else:
                                # C rows land at stack offset Vsp=320: tile
                                # 2+s parts 64:, then tile 3+s parts :ssz-64
                                fh = min(64, ssz)
                                cp(XB[64:64 + fh,
                                      (2 + s) * BF + c0_:(2 + s) * BF + c1_],
                                   pc[0:fh, :cw])
                                if ssz > 64:
                                    cp(XB[0:ssz - 64,
                                          (3 + s) * BF + c0_:(3 + s) * BF + c1_],
                                       pc[64:ssz, :cw])

                if li == 0:
                    # pace LU2p load: gated on c0's first B-linear output so
                    # its transfers neither delay fc2w nor collide with the
                    # BN0 collective's DMA window
                    late_load(LU2p, LU2p_d, 4, 1600, XB[0:1, 0:1])

                # ---- transposed spmm + copies + 512-wide stats windows ----
                XFn = actp.tile([128, nGp * V], bf16, tag=f"XFn{li}")
                FD = nGp * V
                nch = FD // 512
                bnst = miscp.tile([128, nch * 6], f32, tag=f"bnst{li}")
                stat_done = [0, 0]  # cols copied, windows emitted

                def emit_stats(done, XFn=XFn, bnst=bnst, sd=stat_done, nch=nch):
                    sd[0] = done
                    while sd[1] < nch and (sd[1] + 1) * 512 <= sd[0]:
                        ci = sd[1]
                        nc.vector.bn_stats(
                            bnst[:, ci * 6:(ci + 1) * 6],
                            XFn[:, ci * 512:(ci + 1) * 512])
                        sd[1] += 1

                def spmm_group(ps, pbase, gp, w0, wcw):
                    """Accumulate output block (gp, w0:w0+wcw) into ps cols
                    pbase:pbase+wcw (wcw <= 512)."""
                    if li == 0:
                        for half, XS in ((0, XB), (1, XC)):
                            nc.tensor.matmul(
                                ps[:, pbase:pbase + wcw],
                                XS[:80, gp * 128:(gp + 1) * 128],
                                Lr[:80, half * lw + w0:half * lw + w0 + wcw],
                                start=(half == 0), stop=False,
                                skip_group_check=True)
                    else:
                        for st in range(5):
                            nc.tensor.matmul(
                                ps[:, pbase:pbase + wcw],
                                XB[:, st * BF + gp * 128:
                                   st * BF + (gp + 1) * 128],
                                Lr[:, st * lw + w0:st * lw + w0 + wcw],
                                start=(st == 0), stop=False,
                                skip_group_check=True)
                    if cfg.name == "c1":
                        for dl in range(2):
                            nc.tensor.matmul(
                                ps[:, pbase:pbase + wcw],
                                WB(f"A1_{dl}", 128),
                                XF_cur[:, (2 * gp + dl) * Vin + w0:
                                       (2 * gp + dl) * Vin + w0 + wcw],
                                start=False, stop=(dl == 1),
                                skip_group_check=True)
                    else:
                        rhs = XF_cur[:, gp * Vin + w0 // 4:
                                     gp * Vin + w0 // 4 + wcw // 4]
                        rhs = rhs.broadcast_to([128, wcw // 4, 4])
                        nc.tensor.matmul(
                            ps[:, pbase:pbase + wcw], WB(f"A{li}", 128), rhs,
                            start=False, stop=True, skip_group_check=True)

                if V <= 512:
                    # pack 2 gp-blocks per PSUM tile (bank-aligned at 512)
                    for gpp in range(0, nGp, 2):
                        ps = psW.tile([128, 1024], f32, tag="big")
                        for gi in range(2):
                            spmm_group(ps, gi * 512, gpp + gi, 0, V)
                        cp(XFn[:, gpp * V:(gpp + 2) * V]
                           .rearrange("p (u w) -> p u w", w=V),
                           ps[:].rearrange("p (u w) -> p u w", w=512)[:, :, :V],
                           eng=0 if gpp + 2 >= nGp else None)
                        emit_stats((gpp + 2) * V)
                else:
                    # c2: V=1280 per gp -> tiles of 1024 + 256
                    for gp in range(nGp):
                        for w0 in (0, 1024):
                            wcw = min(1024, V - w0)
                            ps = psW.tile([128, 1024], f32, tag="big")
                            for nk in range(0, wcw, 512):
                                sub = min(512, wcw - nk)
                                spmm_group(ps, nk, gp, w0 + nk, sub)
                            cp(XFn[:, gp * V + w0: gp * V + w0 + wcw],
                               ps[:, :wcw],
                               eng=0 if gp == nGp - 1 else None)
                            emit_stats(gp * V + w0 + wcw)

                # ---- BN: partial sums -> AllGather -> scale/shift ----
                n_g = float(B * V)
                aggr = miscp.tile([128, 2], f32, tag="aggr")
                nc.vector.bn_aggr(
                    aggr[:], bnst[:].rearrange("p (c s) -> p c s", s=6))
                part = miscp.tile([128, 2], f32, tag="part")
                nc.vector.tensor_tensor(
                    out=part[:, 1:2], in0=aggr[:, 0:1], in1=aggr[:, 0:1],
                    op=ALU.mult)
                nc.vector.tensor_tensor(
                    out=part[:, 1:2], in0=part[:, 1:2], in1=aggr[:, 1:2],
                    op=ALU.add)
                fscl = float(FD) / n_g
                nc.vector.tensor_scalar_mul(part[:, 1:2], part[:, 1:2], fscl)
                nc.vector.tensor_scalar_mul(part[:, 0:1], aggr[:, 0:1], fscl)
                sel = smalls[:, 0:64] if F == 64 else smalls[:, 64:96]
                pst = pslin.tile([128, 512], f32, tag="lin")
                nc.tensor.matmul(pst[:1, :F], part[:, 0:1], sel,
                                 start=True, stop=True, skip_group_check=True)
                nc.tensor.matmul(pst[:1, F:2 * F], part[:, 1:2], sel,
                                 start=True, stop=True, skip_group_check=True)
                stats_l = miscp.tile([1, 2 * F], f32, tag="statl")
                nc.vector.tensor_copy(stats_l[:], pst[:1, :2 * F])
                bin_ = dramp.tile([1, 2 * F], f32, tag=f"arin{li}")
                bout = dramp.tile([NCORES, 2 * F], f32, tag=f"arout{li}")
                nc.sync.dma_start(bin_[:], stats_l[:])
                nc.gpsimd.collective_compute(
                    "AllGather", ALU.bypass,
                    replica_groups=[list(range(NCORES))],
                    ins=[bin_.opt()], outs=[bout.opt()])
                statg8 = miscp.tile([NCORES, 2 * F], f32, tag="statg8")
                nc.sync.dma_start(statg8[:], bout[:])
                psg = pslin.tile([128, 512], f32, tag="lin")
                nc.tensor.matmul(psg[:1, :2 * F], ones8[:], statg8[:],
                                 start=True, stop=True, skip_group_check=True)
                # s,t from global [mu | Ex^2] (psg)
                st = miscp.tile([1, 2 * F], f32, tag="st")
                tmp = miscp.tile([1, 2 * F], f32, tag="sttmp")
                mu2 = miscp.tile([1, F], f32, tag="mu2")
                nc.vector.tensor_copy(tmp[:, :2 * F], psg[:1, :2 * F])
                nc.vector.tensor_tensor(out=mu2[:], in0=tmp[:, 0:F],
                                        in1=tmp[:, 0:F], op=ALU.mult)
                nc.vector.tensor_tensor(out=tmp[:, F:2 * F],
                                        in0=tmp[:, F:2 * F],
                                        in1=mu2[:], op=ALU.subtract)
                nc.scalar.activation(tmp[:, F:2 * F], tmp[:, F:2 * F],
                                     AF.Sqrt, bias=eps_t[:])
                nc.vector.reciprocal(tmp[:, F:2 * F], tmp[:, F:2 * F])
                nc.vector.tensor_tensor(out=st[:, 0:F],
                                        in0=tmp[:, F:2 * F],
                                        in1=gb_sb[li][:, 0:F], op=ALU.mult)
                nc.vector.tensor_tensor(out=mu2[:], in0=tmp[:, 0:F],
                                        in1=st[:, 0:F], op=ALU.mult)
                nc.vector.tensor_tensor(out=st[:, F:2 * F],
                                        in0=gb_sb[li][:, F:2 * F],
                                        in1=mu2[:], op=ALU.subtract)
                pss = pslin.tile([128, 512], f32, tag="lin", name="pss")
                nc.tensor.transpose(pss[:2 * F, 0:1], st[:],
                                    ident_f[:1, :1])
                stc = miscp.tile([128, 2], f32, tag=f"stc{li}")
                for j in range(cfg.Gp):
                    nc.vector.tensor_copy(stc[j * F:(j + 1) * F, 0:1],
                                          pss[:F, 0:1])
                    nc.vector.tensor_copy(stc[j * F:(j + 1) * F, 1:2],
                                          pss[F:2 * F, 0:1])
                # chunked scale/shift + relu, split across Act/DVE/Pool
                # (contiguous g-aligned chunks: subtile deps track them)
                def relu_chunk(ap, eng):
                    if eng == 1:
                        nc.vector.tensor_scalar(ap, ap, stc[:, 0:1],
                                                stc[:, 1:2], ALU.mult, ALU.add)
                        nc.vector.tensor_scalar_max(ap, ap, 0.0)
                    elif eng == 2:
                        nc.gpsimd.tensor_scalar(ap, ap, stc[:, 0:1],
                                                stc[:, 1:2], ALU.mult, ALU.add)
                        nc.gpsimd.tensor_scalar_max(ap, ap, 0.0)
                    else:
                        nc.scalar.activation(ap, ap, AF.Relu,
                                             scale=stc[:, 0:1],
                                             bias=stc[:, 1:2])

                csz = FD // 8
                # engine shares ~ inverse throughput: Act 4, DVE 3, Pool 1
                sched = [0, 1, 0, 1, 2, 0, 1, 0]
                for rc in range(8):
                    relu_chunk(XFn[:, rc * csz:(rc + 1) * csz], sched[rc])
                if li == 0:
                    late_load(LT2, LT2_d, 8, 1600, XFn[0:1, 0:1])
                elif li == 1:
                    late_load(LL2, LL2_d, 8, 1600, XFn[0:1, 0:1])
                XF_cur = XFn

            # ================= c3 (V-layout) + output =================
            # XF_cur = XF2 [128 (j4,c32), 8*1280], G=4, nG=8.
            # Per-s pipeline: relu chunk s -> B/C linears of source tile s ->
            # contributions of s accumulated into 2 persistent PSUM tiles
            # (t 0-4 and t 5-9), so the spmm overlaps the relu chunks.
            XB3 = actp.tile([128, 5120], bf16, tag="XB",
                            name="XB3")[:, :960]
            XC3 = actp.tile([128, 2048], bf16, tag="XC",
                            name="XC3")[:, :960]
            for s in range(10):
                pc = pslin.tile([128, 512], f32, tag="lin")
                for dst, wnm, off in ((XB3, "B3", 0), (XC3, "C3", 96)):
                    Wt = WB(wnm, 12)
                    for g in range(8):
                        nc.tensor.matmul(
                            pc[:, off + g * 12: off + (g + 1) * 12],
                            XF_cur[:, g * 1280 + s * 128:
                                   g * 1280 + (s + 1) * 128],
                            Wt, start=True, stop=True, skip_group_check=True)
                cp(XB3[:, s * 96:(s + 1) * 96], pc[:, 0:96])
                cp(XC3[:, s * 96:(s + 1) * 96], pc[:, 96:192])
            ysb = miscp.tile([128, 960], f32, tag="ysb")
            for t in range(10):
                pv = psW.tile([128, 1024], f32, tag="big")
                for s in range(10):
                    nc.tensor.matmul(
                        pv[:, 0:96],
                        LT2[:, s * 1280 + t * 128:s * 1280 + (t + 1) * 128],
                        XB3[:, s * 96:(s + 1) * 96],
                        start=(s == 0), stop=False, skip_group_check=True)
                for s in range(10):
                    nc.tensor.matmul(
                        pv[:, 0:96],
                        LL2[:, s * 1280 + t * 128:s * 1280 + (t + 1) * 128],
                        XC3[:, s * 96:(s + 1) * 96],
                        start=False, stop=False, skip_group_check=True)
                for g in range(8):
                    nc.tensor.matmul(
                        pv[:, g * 12:(g + 1) * 12],
                        XF_cur[:, g * 1280 + t * 128:g * 1280 + (t + 1) * 128],
                        WB("A3", 12),
                        start=False, stop=False, skip_group_check=True)
                nc.tensor.matmul(
                    pv[:, 0:96], onesr[:, :128], b3rb[:, 0:96],
                    start=False, stop=True, skip_group_check=True)
                cp(ysb[:, t * 96:(t + 1) * 96], pv[:, 0:96])
            nc.sync.dma_start(ydram[:, 0:480], ysb[:, 0:480])
            nc.sync.dma_start(ydram[:, 480:960], ysb[:, 480:960])

    nc.compile()
    return nc


def kernel(**inputs):
    import sys
    for p in ("/opt/trn_rl_repo", "/opt/trn_rl_repo/concourse"):
        if p not in sys.path:
            sys.path.insert(0, p)
    from concourse.bass_utils import run_bass_kernel_spmd
    import ml_dtypes

    host = _build_host(inputs)
    woffs = host.pop("_woffs")
    xT_full = host.pop("xTp_full")

    key = ("nc",)
    if key not in _CACHE:
        _CACHE[key] = _build_nc(woffs)
    nc = _CACHE[key]

    in_maps = []
    for c in range(NCORES):
        m = dict(host)
        xc = xT_full[:, c * BL:(c + 1) * BL]  # [2048, 32]
        m["xTp"] = np.ascontiguousarray(
            xc.reshape(16, 128, BL).transpose(1, 0, 2).reshape(128, 16 * BL)
        ).astype(ml_dtypes.bfloat16)
        in_maps.append(m)
    res = run_bass_kernel_spmd(nc, in_maps, core_ids=list(range(NCORES)))
    outs = []
    for c in range(NCORES):
        y = res.results[c]["y"].astype(np.float32)  # [128, 960]
        outs.append(y.reshape(128, 10, BL, 3).transpose(2, 1, 0, 3)
                    .reshape(BL, 1280, 3))
    return np.concatenate(outs, axis=0)


if __name__ == "__main__":
    import reference as R
    inp = R.setup_inputs()
    inp = {k: np.asarray(v) for k, v in inp.items()}
    act = kernel(**inp)
    exp = np.asarray(R.reference(**inp))
    err = np.linalg.norm(act - exp) / np.linalg.norm(exp)
    print("Relative error:", err)
